# revision 50
# baseline (speedup 1.0000x reference)
"""Trainium2 Bass kernel for nn_BotAwareGAT (2-layer hetero GAT + MLP).

Strategy (8 NeuronCores, SPMD):
  - Destination-partitioned: core k owns dst nodes [k*2500, (k+1)*2500).
  - Dense projections replicated; per-edge softmax-aggregation uses an ELL
    schedule: per edge type the core's dsts are degree-sorted and packed into
    groups of 64; each 128-edge chunk covers 2 rounds of a group so the
    segment-sum one-hot matrix is a compile-time constant.  Messages are
    fetched with batched dma_gather (4 SWDGE queues round-robin) from a
    node-major table [h | s_src]; attention weights q=exp(lrelu(...)) are
    built on DVE/ACT; the gathered messages are scaled by q in place; one PE
    matmul per chunk accumulates messages per dst, one per piece accumulates
    q; normalization happens per 128-slot batch.
  - Per-dst scores (s_dst) live in tiny local tables built from per-core
    sliced inputs, so score-distribution gathers overlap the table builds.
  - All weight augmentation (attention-vector folds) precomputed on host.
  - Layer-1 results are exchanged with AllGather; padding edges point at a
    poison table row whose s_src = -600 so exp() underflows to 0.
"""

import numpy as np
import ml_dtypes

N = 20000
NCORES = 8
ND = N // NCORES            # 2500 dst nodes per core
GD = 64                     # dsts per group
NSLOT = 2560                # padded slots per core (40 groups)
NG = NSLOT // GD            # 40 groups
HEADS = 8
POISON = N                  # poison row index in big tables
T1C = 256                   # L1 table cols (bf16): [h1(128) | ssrc(8) | pad]
T2C = 640                   # L2 table cols: [h2(512) | ssrc(8) | pad]
NCH = 8                     # max chunks per dma_gather call (1024 idx)
NEG = 0.2

bf16 = ml_dtypes.bfloat16


# ----------------------------------------------------------------------------
# host-side schedule construction
# ----------------------------------------------------------------------------

def _wrap16(a):
    """[L] int -> [128, L//16] int16 (dma_gather/scatter index layout,
    replicated over the 8 q7 partition groups)."""
    w = a.reshape(-1, 16).T.astype(np.int16)
    return np.tile(w, (8, 1))


def _schedule_for_type(src, dst):
    """Build per-core ELL schedules for one edge type."""
    percore = []
    for k in range(NCORES):
        sel = (dst >= k * ND) & (dst < (k + 1) * ND)
        s = src[sel].astype(np.int64)
        d = (dst[sel] - k * ND).astype(np.int64)
        deg = np.bincount(d, minlength=ND)
        order = np.argsort(-deg, kind="stable")
        o = np.argsort(d, kind="stable")
        s_sorted = s[o]
        starts = np.zeros(ND + 1, np.int64)
        np.cumsum(deg, out=starts[1:])
        percore.append((deg, order, s_sorted, starts))

    Rg = np.zeros(NG, np.int64)
    for g in range(NG):
        mx = 1
        lo, hi = g * GD, (g + 1) * GD
        for (deg, order, _, _) in percore:
            real_hi = min(hi, ND)
            if lo < real_hi:
                mx = max(mx, int(deg[order[lo:real_hi]].max()))
        rpc = 128 // GD
        Rg[g] = ((mx + rpc - 1) // rpc) * rpc
    Cg = Rg // (128 // GD)
    cbase = np.zeros(NG, np.int64)
    np.cumsum(Cg[:-1], out=cbase[1:])
    TC = int(Cg.sum())
    TCpad = ((TC + NCH - 1) // NCH) * NCH

    gidx_all, scat_all, sdti_all = [], [], []
    for k in range(NCORES):
        deg, order, s_sorted, starts = percore[k]
        gidx = np.full(TCpad * 128, POISON, np.int64)
        scat = np.zeros(NSLOT, np.int64)
        sdti = np.full(2 * NSLOT, ND, np.int64)   # local poison row = ND
        for g in range(NG):
            base = cbase[g]
            for j in range(GD):
                rank = g * GD + j
                if rank < ND:
                    dd = int(order[rank])
                    scat[rank] = dd
                    sdti[g * 128 + j] = dd
                    sdti[g * 128 + GD + j] = dd
                    dg = int(deg[dd])
                    if dg:
                        r = np.arange(dg)
                        rpc = 128 // GD
                        pos = (base + r // rpc) * 128 + (r % rpc) * GD + j
                        gidx[pos] = s_sorted[starts[dd]:starts[dd] + dg]
                else:
                    scat[rank] = rank   # trash rows 2500..2559
        ncalls = TCpad // NCH
        gidx_all.append(_wrap16(gidx).reshape(128, ncalls, NCH * 8).transpose(1, 0, 2))
        scat_all.append(_wrap16(scat))
        sdti_all.append((_wrap16(sdti), sdti.copy()))

    return dict(Rg=Rg, cbase=cbase, TC=TC, TCpad=TCpad,
                gidx=np.stack(gidx_all), scat=np.stack(scat_all),
                sdti=np.stack([a for a, _ in sdti_all]),
                sdti_raw=np.stack([b for _, b in sdti_all]))


def _fold(a):
    """a [H, C] -> block-diag fold [H*C, H]."""
    H, C = a.shape
    out = np.zeros((H * C, H), np.float32)
    for h in range(H):
        out[h * C:(h + 1) * C, h] = a[h]
    return out


def _host_prep(inputs):
    x = np.asarray(inputs["x"], np.float32)
    W1 = np.asarray(inputs["W1"], np.float32)
    a1s = np.asarray(inputs["a1_src"], np.float32)
    a1d = np.asarray(inputs["a1_dst"], np.float32)
    W2 = np.asarray(inputs["W2"], np.float32)
    a2s = np.asarray(inputs["a2_src"], np.float32)
    a2d = np.asarray(inputs["a2_dst"], np.float32)
    Wc1 = np.asarray(inputs["Wc1"], np.float32)
    Wc2 = np.asarray(inputs["Wc2"], np.float32)

    shared = {}
    shared["xT"] = np.ascontiguousarray(x.T.reshape(2, 128, N)).astype(bf16)

    # Head-interleaved feature order: on-device hidden col j holds original
    # head-major dim perm1[j], i.e. j = c*H + h  <->  orig = h*C + c.  This
    # makes the per-head attention-weight broadcast step-1 on the innermost
    # axis so the DVE multiply runs in 2x mode.
    perm1 = np.array([(j % 8) * 16 + j // 8 for j in range(128)])   # j -> orig

    # L1 weights: w1cat[c] = [128, 288]; per type t: cols [t*144, t*144+128)
    # = W1[t] (cols permuted) rows 128c..128c+128, [+128, +136) = W1@fold(a1s).
    w1cat = np.zeros((2, 128, 288), np.float32)
    w1dst = np.zeros((2, 128, 16), np.float32)
    for t in range(2):
        ws = W1[t] @ _fold(a1s[t])          # [256, 8]
        wd = W1[t] @ _fold(a1d[t])          # [256, 8]
        W1p = W1[t][:, perm1]
        for c in range(2):
            w1cat[c, :, t * 144:t * 144 + 128] = W1p[c * 128:(c + 1) * 128]
            w1cat[c, :, t * 144 + 128:t * 144 + 136] = ws[c * 128:(c + 1) * 128]
            w1dst[c, :, t * 8:(t + 1) * 8] = wd[c * 128:(c + 1) * 128]
    shared["w1cat"] = w1cat.astype(bf16)
    shared["w1dst"] = w1dst.astype(bf16)

    # L2 weights: rows permuted by perm1 (h_mid cols are permuted), msg cols
    # permuted by perm2 (c*H + h).  w2cat [128, 1040]: [0:512] W2a, [512:1024]
    # W2b, [1024:1032] a-ssrc fold, [1032:1040] b-ssrc fold.  w2dst [128, 16].
    perm2 = np.array([(j % 8) * 64 + j // 8 for j in range(512)])
    w2cat = np.zeros((128, 1040), np.float32)
    w2dst = np.zeros((128, 16), np.float32)
    for t in range(2):
        w2cat[:, t * 512:(t + 1) * 512] = W2[t][perm1][:, perm2]
        w2cat[:, 1024 + t * 8:1032 + t * 8] = (W2[t] @ _fold(a2s[t]))[perm1]
        w2dst[:, t * 8:(t + 1) * 8] = (W2[t] @ _fold(a2d[t]))[perm1]
    shared["w2cat"] = w2cat.astype(bf16)
    shared["w2dst"] = w2dst.astype(bf16)

    shared["wc1"] = Wc1.astype(bf16)
    shared["wc2"] = Wc2.astype(bf16)

    S = np.zeros((128, GD), np.float32)
    for e in range(128):
        S[e, e % GD] = 1.0
    shared["sconst"] = S.astype(bf16)
    shared["ident"] = np.eye(128, dtype=np.float32).astype(bf16)
    p1 = np.zeros((1, T1C), np.float32)
    p1[0, 128:136] = -600.0
    shared["poison1"] = p1.astype(bf16)
    p2 = np.zeros((1, T2C), np.float32)
    p2[0, 512:520] = -600.0
    shared["poison2"] = p2.astype(bf16)

    ei_a = np.asarray(inputs["edge_index_a"])
    ei_b = np.asarray(inputs["edge_index_b"])
    sched_a = _schedule_for_type(ei_a[0], ei_a[1])
    sched_b = _schedule_for_type(ei_b[0], ei_b[1])

    per_core = []
    for k in range(NCORES):
        m = dict(shared)
        xds = np.zeros((2, 2, 128, 2 * NSLOT), np.float32)
        for t, sch in ((0, sched_a), (1, sched_b)):
            dd = sch["sdti_raw"][k]
            rows = np.where(dd == ND, k * ND, k * ND + dd)
            m_t = x[rows].T                      # [256, 2*NSLOT]
            xds[t] = m_t.reshape(2, 128, 2 * NSLOT)
        m["xds"] = xds.astype(bf16)
        m["gidx_a"] = sched_a["gidx"][k]
        m["gidx_b"] = sched_b["gidx"][k]
        m["scat_a"] = sched_a["scat"][k]
        m["scat_b"] = sched_b["scat"][k]
        m["sdti_a"] = sched_a["sdti"][k]
        m["sdti_b"] = sched_b["sdti"][k]
        per_core.append(m)
    return per_core, sched_a, sched_b


# ----------------------------------------------------------------------------
# device kernel
# ----------------------------------------------------------------------------

def _patch_queue_aware_lanes():
    """Make Tile's SWDGE DMA semaphore-lane assignment queue-aware: queue q
    gets lanes {2q, 2q+1}."""
    import concourse.tile_sem_assignment as tsa
    import concourse.mybir as mybir
    if getattr(tsa, "_qaware_patched", False):
        return
    orig = tsa.TileClockTick._assign_tick

    def patched(self, inst):
        if (isinstance(inst, tsa.DMAInst)
                and inst.engine == mybir.EngineType.Pool
                and not isinstance(inst, tsa.bass_isa.UserSyncedRemoteDMADescs)):
            q = getattr(inst, "queue_num", 0) or 0
            tog = getattr(self, "_q_toggle", None)
            if tog is None:
                tog = self._q_toggle = {}
            self.next_sw_dma_idx = (q * 2 + tog.get(q, 0)) % self.swdge_sem_count
            tog[q] = 1 - tog.get(q, 0)
        return orig(self, inst)

    tsa.TileClockTick._assign_tick = patched
    tsa._qaware_patched = True


def _build_nc(sched_a, sched_b):
    import concourse.bacc as bacc
    import concourse.mybir as mybir
    import concourse.tile as tile

    _patch_queue_aware_lanes()

    BF = mybir.dt.bfloat16
    F32 = mybir.dt.float32
    F8 = mybir.dt.float8e4
    I16 = mybir.dt.int16
    AF = mybir.ActivationFunctionType
    OP = mybir.AluOpType
    AX = mybir.AxisListType

    nc = bacc.Bacc("TRN2", target_bir_lowering=False, debug=False,
                   num_devices=NCORES, num_swdge_queues=4)

    def din(name, shape, dt=BF):
        return nc.dram_tensor(name, shape, dt, kind="ExternalInput").ap()

    scheds = {"a": sched_a, "b": sched_b}
    NSC = NSLOT // 16
    NCC = NSLOT // 128

    xT = din("xT", [2, 128, N])
    xds = din("xds", [2, 2, 128, 2 * NSLOT])
    w1cat = din("w1cat", [2, 128, 288])
    w1dst = din("w1dst", [2, 128, 16])
    w2cat = din("w2cat", [128, 1040])
    w2dst = din("w2dst", [128, 16])
    wc1 = din("wc1", [64, 32])
    wc2 = din("wc2", [32, 2])
    sconst = din("sconst", [128, GD])
    ident = din("ident", [128, 128])
    poison1 = din("poison1", [1, T1C])
    poison2 = din("poison2", [1, T2C])
    gidx_d = {t: din(f"gidx_{t}", [scheds[t]["TCpad"] // NCH, 128, NCH * 8], I16)
              for t in "ab"}
    scat_d = {t: din(f"scat_{t}", [128, NSC], I16) for t in "ab"}
    sdti_d = {t: din(f"sdti_{t}", [128, 2 * NSC], I16) for t in "ab"}
    out = nc.dram_tensor("out", [ND, 2], F32, kind="ExternalOutput").ap()

    qrr_state = {"a": 0, "b": 0, None: 0}

    def qrr(t=None):
        i = qrr_state[t]
        qrr_state[t] = 1 - i
        if t is None:
            q = qrr_state["__g"] = (qrr_state.get("__g", -1) + 1) % 4
            return q
        return (0 if t == "a" else 2) + i

    with tile.TileContext(nc) as tc:
        with tc.tile_pool(name="dram", bufs=1, space="DRAM") as dpool, \
             tc.tile_pool(name="const", bufs=1) as cpool:

            table1cat = dpool.tile([2, N + 1, T1C], BF, tag="tb1")
            table2cat = dpool.tile([2, N + 1, T2C], BF, tag="tb2")
            # local per-core dst-score tables [ND+1, 128]: cols 0:16 scores
            sdtab2 = dpool.tile([ND + 1, 128], BF, tag="sdtab2")
            acc1 = dpool.tile([NSLOT, 128], F32, tag="acc1")
            acc2 = dpool.tile([NSLOT, 64], F32, tag="acc2")
            # AllGather chunking: 2 column chunks of <=1280 local nodes
            AGC = 1280
            NAG = 2
            h2sliceT = dpool.tile([NAG, 128, AGC], BF, tag="h2sT")
            h2fullT = dpool.tile([NAG, NCORES, 128, AGC], BF, tag="h2fT")

            # ---- constants ----
            sconst_sb = cpool.tile([128, GD], BF)
            nc.sync.dma_start(sconst_sb[:], sconst[:])
            id_sb = cpool.tile([128, 128], BF)
            nc.sync.dma_start(id_sb[:], ident[:])
            wc1_sb = cpool.tile([64, 32], BF)
            nc.sync.dma_start(wc1_sb[:], wc1[:])
            wc2_sb = cpool.tile([32, 2], BF)
            nc.sync.dma_start(wc2_sb[:], wc2[:])
            w1cat_sb = cpool.tile([128, 2, 288], BF)
            nc.sync.dma_start(w1cat_sb[:], w1cat.rearrange("c p f -> p c f"))
            w1dst_sb = cpool.tile([128, 2, 16], BF)
            nc.sync.dma_start(w1dst_sb[:], w1dst.rearrange("c p f -> p c f"))
            w2cat_sb = cpool.tile([128, 1040], BF)
            nc.sync.dma_start(w2cat_sb[:], w2cat[:])
            w2dst_sb = cpool.tile([128, 16], BF)
            nc.sync.dma_start(w2dst_sb[:], w2dst[:])
            scat_sb = {}
            sdti_sb = {}
            for t in "ab":
                scat_sb[t] = cpool.tile([128, NSC], I16, tag=f"scat{t}", name=f"scatsb{t}")
                nc.sync.dma_start(scat_sb[t][:], scat_d[t][:])
                sdti_sb[t] = cpool.tile([128, 2 * NSC], I16, tag=f"sdti{t}", name=f"sdtisb{t}")
                nc.sync.dma_start(sdti_sb[t][:], sdti_d[t][:])
            gidx_sb = {}
            for t in "ab":
                ncalls = scheds[t]["TCpad"] // NCH
                gidx_sb[t] = cpool.tile([128, ncalls, NCH * 8], I16,
                                        tag=f"gidx{t}", name=f"gidxsb{t}")
                nc.scalar.dma_start(gidx_sb[t][:],
                                    gidx_d[t].rearrange("c p s -> p c s"))

            for ti in range(2):
                nc.sync.dma_start(table1cat[ti, N:N + 1, :], poison1[:])
                nc.sync.dma_start(table2cat[ti, N:N + 1, :], poison2[:])

            NT = (ND + 127) // 128    # 20 tiles of local dst rows

            # Tiny dummy AllGather fired during the (idle-CC) head phase so the
            # one-time cross-core rendezvous barrier cost is absorbed here
            # instead of serializing the real collectives in the midsection.
            dmy_in = dpool.tile([128, 16], BF, tag="dmyi")
            dmy_out = dpool.tile([NCORES, 128, 16], BF, tag="dmyo")
            with tc.tile_pool(name="dmy", bufs=1) as dmyp:
                dz = dmyp.tile([128, 16], BF)
                nc.vector.memset(dz[:], 0.0)
                nc.sync.dma_start(dmy_in[:], dz[:])
            nc.gpsimd.collective_compute(
                "AllGather", mybir.AluOpType.bypass,
                replica_groups=[list(range(NCORES))],
                ins=[dmy_in.opt()], outs=[dmy_out.opt()])

            # ---- zero accumulators + sdtab poison rows ----
            with tc.tile_pool(name="zacc", bufs=1) as zaccp:
                zt = zaccp.tile([128, NCC, 128], F32)
                nc.vector.memset(zt[:], 0.0)
                nc.sync.dma_start(acc1.rearrange("(a p) c -> p a c", p=128), zt[:])
                nc.sync.dma_start(acc2.rearrange("(a p) c -> p a c", p=128),
                                  zt[:, :, 0:64])
                zb = zaccp.tile([1, 128], BF)
                nc.vector.memset(zb[:], 0.0)
                nc.sync.dma_start(sdtab2[ND:ND + 1, :], zb[:])

            # ---- sdt1s: slot-ordered dst scores for L1, computed straight
            # into SBUF (no DRAM round trip, no gathers) ----
            sdt1s = cpool.tile([128, NG, 16], BF, tag="sdt1s")
            with tc.tile_pool(name="sd1", bufs=3) as sd1p, \
                 tc.tile_pool(name="sd1ps", bufs=2, space="PSUM") as sd1ps:
                xds_sb = sd1p.tile([128, 2, 2, 2 * NSLOT], BF, tag="xd", bufs=1)
                nc.scalar.dma_start(xds_sb[:],
                                    xds.rearrange("t c p s -> p t c s"))
                for i in range(2 * NSLOT // 128):
                    lo = i * 128
                    for ti in range(2):
                        ps = sd1ps.tile([128, 8], F32, tag="sd1ps")
                        nc.tensor.matmul(
                            out=ps[:], lhsT=xds_sb[:, ti, 0, lo:lo + 128],
                            rhs=w1dst_sb[:, 0, ti * 8:ti * 8 + 8],
                            start=True, stop=False)
                        nc.tensor.matmul(
                            out=ps[:], lhsT=xds_sb[:, ti, 1, lo:lo + 128],
                            rhs=w1dst_sb[:, 1, ti * 8:ti * 8 + 8],
                            start=False, stop=True)
                        nc.vector.tensor_copy(
                            out=sdt1s[:, i, ti * 8:ti * 8 + 8], in_=ps[:])

            # ---- phase 1: layer-1 tables (types fused) ----
            with tc.tile_pool(name="ph1", bufs=4) as p1p, \
                 tc.tile_pool(name="ph1ps", bufs=3, space="PSUM") as p1ps:
                xt_sb = [p1p.tile([128, N], BF, tag=f"xt{c}", name=f"xtsb{c}", bufs=1) for c in range(2)]
                nc.scalar.dma_start(xt_sb[0][:], xT[0])
                nc.sync.dma_start(xt_sb[1][:], xT[1])
                for i in range((N + 127) // 128):
                    lo = i * 128
                    m = min(128, N - lo)
                    ps = p1ps.tile([128, 288], F32, tag="t1ps")
                    nc.tensor.matmul(out=ps[:m], lhsT=xt_sb[0][:, lo:lo + m],
                                     rhs=w1cat_sb[:, 0, :], start=True, stop=False)
                    nc.tensor.matmul(out=ps[:m], lhsT=xt_sb[1][:, lo:lo + m],
                                     rhs=w1cat_sb[:, 1, :], start=False, stop=True)
                    o = p1p.tile([128, 288], BF, tag="t1o")
                    if i % 2 == 0:
                        nc.scalar.copy(out=o[:m], in_=ps[:m])
                    else:
                        nc.vector.tensor_copy(out=o[:m], in_=ps[:m])
                    eng = nc.sync if i % 2 == 0 else nc.scalar
                    eng.dma_start(
                        table1cat[:, lo:lo + m, 0:144].rearrange(
                            "t p f -> p t f"),
                        o[:m].rearrange("p (t f) -> p t f", t=2))

            # ---- edge phase ----
            def gather_sdt(layer, pool, sdtab):
                sdt = {}
                for t in "ab":
                    sdt[t] = pool.tile([128, NG, 128], BF, tag=f"sdt{t}",
                                       name=f"sdt{layer}{t}", bufs=1)
                    sdone = 0
                    while sdone < 2 * NSLOT:
                        n = min(1024, 2 * NSLOT - sdone)
                        nc.gpsimd.dma_gather(
                            sdt[t][:, sdone // 128:(sdone + n) // 128, :],
                            sdtab[:],
                            sdti_sb[t][:, sdone // 16:(sdone + n) // 16],
                            n, n, 128, elem_step=128, queue_num=qrr(t))
                        sdone += n
                return sdt

            def edge_phase(layer, tables, parks, sdt, acc, gbufs):
                CT = T1C if layer == 1 else T2C
                C = 128 if layer == 1 else 512
                SC = C
                hb = C // 8
                NV = 128 // GD

                with tc.tile_pool(name=f"eg{layer}", bufs=gbufs) as gp, \
                     tc.tile_pool(name=f"ew{layer}", bufs=4) as wp, \
                     tc.tile_pool(name=f"es{layer}", bufs=6) as sp, \
                     tc.tile_pool(name=f"ef{layer}", bufs=2) as fp, \
                     tc.tile_pool(name=f"eps{layer}", bufs=2, space="PSUM") as pp, \
                     tc.tile_pool(name=f"ezs{layer}", bufs=2, space="PSUM") as zp:

                    st = {t: dict(call=-1, G=None, Wb=None, pa=None, pz=None)
                          for t in "ab"}

                    def do_group(t, g):
                        ti = 0 if t == "a" else 1
                        sched = scheds[t]
                        cg = int(sched["Rg"][g] // (128 // GD))
                        base = int(sched["cbase"][g])
                        s_ = st[t]
                        if g % NV == 0:
                            s_["pa"] = pp.tile([128, C], F32, tag=f"pa{t}",
                                               name=f"pa{layer}{t}")
                            s_["pz"] = zp.tile([128, NCH, 8], F32, tag=f"pz{t}",
                                               name=f"pz{layer}{t}")
                            nc.vector.memset(s_["pz"][:], 0.0)
                        pa, pz = s_["pa"], s_["pz"]
                        row0 = GD * (g % NV)
                        done = 0
                        while done < cg:
                            seg = min(NCH - (base + done) % NCH, cg - done)
                            call = (base + done) // NCH
                            coff = (base + done) % NCH
                            if call != s_["call"]:
                                G = gp.tile([128, NCH, CT], BF, tag=f"G{t}",
                                            name=f"G{layer}{t}")
                                nc.gpsimd.dma_gather(
                                    G[:, :, :], tables[t][:],
                                    gidx_sb[t][:, call, :],
                                    NCH * 128, NCH * 128, CT,
                                    queue_num=qrr(t))
                                s_["call"] = call
                                s_["G"] = G
                            G = s_["G"]
                            M = G
                            sl = slice(coff, coff + seg)
                            sview = G[:, sl, SC:SC + 8]
                            u = sp.tile([128, NCH, 8], F32, tag=f"u{t}",
                                        name=f"u{layer}{t}")
                            nc.vector.tensor_tensor(
                                out=u[:, :seg, :], in0=sview,
                                in1=sdt[t][:, g, ti * 8:ti * 8 + 8][:, None, :]
                                    .to_broadcast([128, seg, 8]),
                                op=OP.add)
                            phi = sp.tile([128, NCH, 8], F32, tag=f"phi{t}",
                                          name=f"phi{layer}{t}")
                            nc.vector.scalar_tensor_tensor(
                                out=phi[:, :seg, :], in0=u[:, :seg, :], scalar=NEG,
                                in1=u[:, :seg, :], op0=OP.mult, op1=OP.max)
                            q = sp.tile([128, NCH, 8], BF, tag=f"q{t}",
                                        name=f"q{layer}{t}")
                            nc.scalar.activation(out=q[:, :seg, :],
                                                 in_=phi[:, :seg, :], func=AF.Exp)
                            nc.vector.tensor_tensor(
                                out=M[:, sl, 0:C].rearrange(
                                    "p s (c h) -> p s c h", h=8),
                                in0=M[:, sl, 0:C].rearrange(
                                    "p s (c h) -> p s c h", h=8),
                                in1=q[:, :seg, None, :].to_broadcast(
                                    [128, seg, hb, 8]),
                                op=OP.mult)
                            nc.tensor.matmul(
                                out=pz[row0:row0 + GD, :seg, :],
                                lhsT=sconst_sb[:], rhs=q[:, :seg, :],
                                start=False, stop=(done + seg == cg),
                                skip_group_check=True)
                            for s in range(seg):
                                cc = done + s
                                nc.tensor.matmul(
                                    out=pa[row0:row0 + GD, :],
                                    lhsT=sconst_sb[:], rhs=M[:, coff + s, 0:C],
                                    start=(cc == 0), stop=(cc == cg - 1),
                                    skip_group_check=True)
                            done += seg
                        if g % NV == NV - 1:
                            mi = (g * GD) // 128
                            zs = sp.tile([128, 8], F32, tag=f"zs{t}",
                                         name=f"zs{layer}{t}")
                            nc.vector.tensor_reduce(
                                out=zs[:, :, None],
                                in_=pz.rearrange("p s h -> p h s"),
                                axis=AX.X, op=OP.add)
                            z8 = sp.tile([128, 8], F32, tag=f"z8{t}",
                                         name=f"z8{layer}{t}")
                            nc.vector.tensor_scalar(
                                out=z8[:], in0=zs[:],
                                scalar1=(1.0 if layer == 1 else 8.0),
                                scalar2=1e-30, op0=OP.mult, op1=OP.max)
                            rz = sp.tile([128, 8], F32, tag=f"rz{t}",
                                         name=f"rz{layer}{t}")
                            nc.vector.reciprocal(out=rz[:], in_=z8[:])
                            if layer == 1:
                                nc.vector.tensor_tensor(
                                    out=parks[t][:, mi, :].rearrange(
                                        "p (c h) -> p c h", h=8),
                                    in0=pa[:, 0:128].rearrange(
                                        "p (c h) -> p c h", h=8),
                                    in1=rz[:, None, :].to_broadcast(
                                        [128, 16, 8]),
                                    op=OP.mult)
                            else:
                                tmp = fp.tile([128, 512], F32, tag=f"tmp{t}",
                                              name=f"tmp{layer}{t}")
                                nc.vector.tensor_tensor(
                                    out=tmp[:].rearrange("p (c h) -> p c h", h=8),
                                    in0=pa[:].rearrange("p (c h) -> p c h", h=8),
                                    in1=rz[:, None, :].to_broadcast(
                                        [128, 64, 8]),
                                    op=OP.mult)
                                nc.vector.tensor_reduce(
                                    out=parks[t][:, mi, :, None],
                                    in_=tmp[:].rearrange("p (c h) -> p c h", h=8),
                                    axis=AX.X, op=OP.add)

                    CA = 128 if layer == 1 else 64
                    windows = {15: (0, 8), 31: (8, 8), 39: (16, 4)}
                    for g in range(NG):
                        for t in "ab":
                            do_group(t, g)
                        if g in windows:
                            c0, nb = windows[g]
                            for t in "ab":
                                nc.gpsimd.dma_scatter_add(
                                    acc[:], parks[t][:, c0:c0 + nb, :],
                                    scat_sb[t][:, c0 * 8:(c0 + nb) * 8],
                                    nb * 128, nb * 128, CA, queue_num=qrr(t))

            # ---- layer-1 edges ----
            with tc.tile_pool(name="park1", bufs=1) as parkp:
                parks = {t: parkp.tile([128, NCC, 128], F32, tag=f"park{t}",
                                       name=f"park1{t}") for t in "ab"}
                edge_phase(1, {"a": table1cat[0], "b": table1cat[1]}, parks,
                           {"a": sdt1s, "b": sdt1s}, acc1, gbufs=8)

            # ---- combine + ELU helper ----
            def elu_combine(src_ap, cols, tilepool, dst_write):
                for i in range(NT):
                    lo = i * 128
                    m = min(128, ND - lo)
                    a = tilepool.tile([128, cols], F32, tag="ec_a")
                    nc.sync.dma_start(a[:m], src_ap[lo:lo + m, :])
                    e = tilepool.tile([128, cols], F32, tag="ec_e")
                    nc.scalar.activation(out=e[:m], in_=a[:m], func=AF.Exp, scale=0.5)
                    em1 = tilepool.tile([128, cols], F32, tag="ec_em1")
                    nc.vector.tensor_scalar(out=em1[:m], in0=e[:m], scalar1=-1.0,
                                            scalar2=None, op0=OP.add)
                    xm = tilepool.tile([128, cols], F32, tag="ec_xm")
                    nc.vector.tensor_scalar(out=xm[:m], in0=a[:m], scalar1=0.5,
                                            scalar2=None, op0=OP.mult)
                    mk = tilepool.tile([128, cols], mybir.dt.uint8, tag="ec_mk")
                    nc.vector.tensor_scalar(out=mk[:m], in0=a[:m], scalar1=0.0,
                                            scalar2=None, op0=OP.is_gt)
                    h = tilepool.tile([128, cols], BF, tag="ec_h")
                    nc.vector.select(out=h[:m], mask=mk[:m], on_true=xm[:m],
                                     on_false=em1[:m])
                    dst_write(i, lo, m, h)

            # L1 combine -> transposed slice; also L2 dst scores -> sdtab2;
            # AllGather fires per finished column chunk.
            with tc.tile_pool(name="elu1", bufs=4) as elup, \
                 tc.tile_pool(name="elu1ps", bufs=3, space="PSUM") as elups:
                def wr1(i, lo, m, h):
                    c = i // 10
                    off = (i % 10) * 128
                    tps = elups.tile([128, 128], BF, tag="e_tp")
                    nc.tensor.transpose(out=tps[:, :m], in_=h[:m, :],
                                        identity=id_sb[:m, :m])
                    ht = elup.tile([128, 128], BF, tag="e_ht")
                    nc.scalar.copy(out=ht[:, :m], in_=tps[:, :m])
                    nc.scalar.dma_start(h2sliceT[c, :, off:off + m], ht[:, :m])
                    ps2 = elups.tile([128, 16], F32, tag="e_sd2")
                    nc.tensor.matmul(out=ps2[:m], lhsT=ht[:, :m],
                                     rhs=w2dst_sb[:], start=True, stop=True)
                    o2 = elup.tile([128, 16], BF, tag="e_sd2o")
                    nc.vector.tensor_copy(out=o2[:m], in_=ps2[:m])
                    nc.scalar.dma_start(sdtab2[lo:lo + m, 0:16], o2[:m])
                    if i % 10 == 9 or i == NT - 1:
                        nc.gpsimd.collective_compute(
                            "AllGather", mybir.AluOpType.bypass,
                            replica_groups=[list(range(NCORES))],
                            ins=[h2sliceT[c].opt()], outs=[h2fullT[c].opt()])
                elu_combine(acc1[:, :], 128, elup, wr1)

            sdt2ctx = tc.tile_pool(name="sdt2p", bufs=1)
            sdt2p = sdt2ctx.__enter__()
            sdt2 = gather_sdt(2, sdt2p, sdtab2)

            # ---- phase 4: layer-2 tables from SBUF-resident h2T (fused) ----
            with tc.tile_pool(name="ph4", bufs=6) as p4p, \
                 tc.tile_pool(name="ph4ps", bufs=3, space="PSUM") as p4ps, \
                 tc.tile_pool(name="ph4px", bufs=2, space="PSUM") as p4px:
                for c in range(NAG):
                    h2t_sb = p4p.tile([128, NCORES, AGC], BF, tag="h2t",
                                      name="h2tsb", bufs=2)
                    nc.sync.dma_start(h2t_sb[:],
                                      h2fullT[c].rearrange("k p j -> p k j"))
                    ctiles = min(10, (ND - c * AGC + 127) // 128)
                    for k8 in range(NCORES):
                        for jj in range(ctiles):
                            off = jj * 128
                            m = min(128, ND - c * AGC - off)
                            row = k8 * ND + c * AGC + off
                            lhs = h2t_sb[:, k8, off:off + m]
                            psa = p4ps.tile([128, 512], F32, tag="t2psa")
                            nc.tensor.matmul(out=psa[:m], lhsT=lhs,
                                             rhs=w2cat_sb[:, 0:512],
                                             start=True, stop=True)
                            psb = p4ps.tile([128, 512], F32, tag="t2psb")
                            nc.tensor.matmul(out=psb[:m], lhsT=lhs,
                                             rhs=w2cat_sb[:, 512:1024],
                                             start=True, stop=True)
                            psx = p4px.tile([128, 16], F32, tag="t2psx")
                            nc.tensor.matmul(out=psx[:m], lhsT=lhs,
                                             rhs=w2cat_sb[:, 1024:1040],
                                             start=True, stop=True)
                            o2 = p4p.tile([128, 2, 520], BF, tag="t2o")
                            nc.scalar.copy(out=o2[:m, 0, 0:512], in_=psa[:m])
                            nc.vector.tensor_copy(out=o2[:m, 1, 0:512],
                                                  in_=psb[:m])
                            nc.scalar.copy(out=o2[:m, 0, 512:520],
                                           in_=psx[:m, 0:8])
                            nc.vector.tensor_copy(out=o2[:m, 1, 512:520],
                                                  in_=psx[:m, 8:16])
                            eng = nc.sync if jj % 2 == 0 else nc.scalar
                            eng.dma_start(
                                table2cat[:, row:row + m, 0:520].rearrange(
                                    "t p f -> p t f"),
                                o2[:m])

            # ---- layer-2 edges ----
            with tc.tile_pool(name="park2", bufs=1) as park2p:
                parks = {t: park2p.tile([128, NCC, 64], F32, tag=f"park2{t}",
                                        name=f"park2{t}") for t in "ab"}
                edge_phase(2, {"a": table2cat[0], "b": table2cat[1]}, parks,
                           sdt2, acc2, gbufs=6)
            sdt2ctx.__exit__(None, None, None)

            # ---- classifier ----
            with tc.tile_pool(name="cls", bufs=4) as clsp, \
                 tc.tile_pool(name="clsps", bufs=2, space="PSUM") as clsps:
                def wrc(i, lo, m, h):
                    tps = clsps.tile([64, 128], BF, tag="c_t1")
                    nc.tensor.transpose(out=tps[:, :m], in_=h[:m, :],
                                        identity=id_sb[:m, :m])
                    h3t = clsp.tile([64, 128], BF, tag="c_h3t")
                    nc.scalar.copy(out=h3t[:, :m], in_=tps[:, :m])
                    z1 = clsps.tile([128, 32], F32, tag="c_z1")
                    nc.tensor.matmul(out=z1[:m], lhsT=h3t[:, :m], rhs=wc1_sb[:],
                                     start=True, stop=True)
                    z1s = clsp.tile([128, 32], BF, tag="c_z1s")
                    nc.scalar.activation(out=z1s[:m], in_=z1[:m], func=AF.Relu)
                    t2ps = clsps.tile([32, 128], BF, tag="c_t2")
                    nc.tensor.transpose(out=t2ps[:, :m], in_=z1s[:m, :],
                                        identity=id_sb[:m, :m])
                    z1t = clsp.tile([32, 128], BF, tag="c_z1t")
                    nc.scalar.copy(out=z1t[:, :m], in_=t2ps[:, :m])
                    lg = clsps.tile([128, 2], F32, tag="c_lg")
                    nc.tensor.matmul(out=lg[:m], lhsT=z1t[:, :m], rhs=wc2_sb[:],
                                     start=True, stop=True)
                    lo_ = clsp.tile([128, 2], F32, tag="c_out")
                    nc.vector.tensor_copy(out=lo_[:m], in_=lg[:m])
                    nc.sync.dma_start(out[lo:lo + m, :], lo_[:m])
                elu_combine(acc2[:, :], 64, clsp, wrc)

    nc.compile()
    return nc


# ----------------------------------------------------------------------------
# entry point
# ----------------------------------------------------------------------------

_CACHE = {}


def _prepare(inputs):
    per_core, sched_a, sched_b = _host_prep(inputs)
    key = (sched_a["TCpad"], sched_b["TCpad"],
           tuple(sched_a["Rg"]), tuple(sched_b["Rg"]))
    if key not in _CACHE:
        _CACHE.clear()
        _CACHE[key] = _build_nc(sched_a, sched_b)
    return _CACHE[key], per_core


def _run(nc, per_core, **kw):
    from concourse import bass_utils
    return bass_utils.run_bass_kernel_spmd(nc, per_core,
                                           core_ids=list(range(NCORES)), **kw)


def kernel(**inputs):
    nc, per_core = _prepare(inputs)
    res = _run(nc, per_core)
    return np.concatenate([res.results[k]["out"] for k in range(NCORES)], 0)


# revision 51
# speedup vs baseline: 1.0282x; 1.0282x over previous
"""Trainium2 Bass kernel for nn_BotAwareGAT (2-layer hetero GAT + MLP).

Strategy (8 NeuronCores, SPMD):
  - Destination-partitioned: core k owns dst nodes [k*2500, (k+1)*2500).
  - Dense projections replicated; per-edge softmax-aggregation uses an ELL
    schedule: per edge type the core's dsts are degree-sorted and packed into
    groups of 64; each 128-edge chunk covers 2 rounds of a group so the
    segment-sum one-hot matrix is a compile-time constant.  Messages are
    fetched with batched dma_gather (4 SWDGE queues round-robin) from a
    node-major table [h | s_src]; attention weights q=exp(lrelu(...)) are
    built on DVE/ACT; the gathered messages are scaled by q in place; one PE
    matmul per chunk accumulates messages per dst, one per piece accumulates
    q; normalization happens per 128-slot batch.
  - Per-dst scores (s_dst) live in tiny local tables built from per-core
    sliced inputs, so score-distribution gathers overlap the table builds.
  - All weight augmentation (attention-vector folds) precomputed on host.
  - Layer-1 results are exchanged with AllGather; padding edges point at a
    poison table row whose s_src = -600 so exp() underflows to 0.
"""

import numpy as np
import ml_dtypes

N = 20000
NCORES = 8
ND = N // NCORES            # 2500 dst nodes per core
GD = 64                     # dsts per group
NSLOT = 2560                # padded slots per core (40 groups)
NG = NSLOT // GD            # 40 groups
HEADS = 8
POISON = N                  # poison row index in big tables
T1C = 256                   # L1 table cols (bf16): [h1(128) | ssrc(8) | pad]
T2C = 640                   # L2 table cols: [h2(512) | ssrc(8) | pad]
NCH = 8                     # max chunks per dma_gather call (1024 idx)
NEG = 0.2

bf16 = ml_dtypes.bfloat16


# ----------------------------------------------------------------------------
# host-side schedule construction
# ----------------------------------------------------------------------------

def _wrap16(a):
    """[L] int -> [128, L//16] int16 (dma_gather/scatter index layout,
    replicated over the 8 q7 partition groups)."""
    w = a.reshape(-1, 16).T.astype(np.int16)
    return np.tile(w, (8, 1))


def _schedule_for_type(src, dst):
    """Build per-core ELL schedules for one edge type."""
    percore = []
    for k in range(NCORES):
        sel = (dst >= k * ND) & (dst < (k + 1) * ND)
        s = src[sel].astype(np.int64)
        d = (dst[sel] - k * ND).astype(np.int64)
        deg = np.bincount(d, minlength=ND)
        order = np.argsort(-deg, kind="stable")
        o = np.argsort(d, kind="stable")
        s_sorted = s[o]
        starts = np.zeros(ND + 1, np.int64)
        np.cumsum(deg, out=starts[1:])
        percore.append((deg, order, s_sorted, starts))

    Rg = np.zeros(NG, np.int64)
    for g in range(NG):
        mx = 1
        lo, hi = g * GD, (g + 1) * GD
        for (deg, order, _, _) in percore:
            real_hi = min(hi, ND)
            if lo < real_hi:
                mx = max(mx, int(deg[order[lo:real_hi]].max()))
        rpc = 128 // GD
        Rg[g] = ((mx + rpc - 1) // rpc) * rpc
    Cg = Rg // (128 // GD)
    cbase = np.zeros(NG, np.int64)
    np.cumsum(Cg[:-1], out=cbase[1:])
    TC = int(Cg.sum())
    TCpad = ((TC + NCH - 1) // NCH) * NCH

    gidx_all, scat_all, sdti_all = [], [], []
    for k in range(NCORES):
        deg, order, s_sorted, starts = percore[k]
        gidx = np.full(TCpad * 128, POISON, np.int64)
        scat = np.zeros(NSLOT, np.int64)
        sdti = np.full(2 * NSLOT, ND, np.int64)   # local poison row = ND
        for g in range(NG):
            base = cbase[g]
            for j in range(GD):
                rank = g * GD + j
                if rank < ND:
                    dd = int(order[rank])
                    scat[rank] = dd
                    sdti[g * 128 + j] = dd
                    sdti[g * 128 + GD + j] = dd
                    dg = int(deg[dd])
                    if dg:
                        r = np.arange(dg)
                        rpc = 128 // GD
                        pos = (base + r // rpc) * 128 + (r % rpc) * GD + j
                        gidx[pos] = s_sorted[starts[dd]:starts[dd] + dg]
                else:
                    scat[rank] = rank   # trash rows 2500..2559
        ncalls = TCpad // NCH
        gidx_all.append(_wrap16(gidx).reshape(128, ncalls, NCH * 8).transpose(1, 0, 2))
        scat_all.append(_wrap16(scat))
        sdti_all.append((_wrap16(sdti), sdti.copy()))

    return dict(Rg=Rg, cbase=cbase, TC=TC, TCpad=TCpad,
                gidx=np.stack(gidx_all), scat=np.stack(scat_all),
                sdti=np.stack([a for a, _ in sdti_all]),
                sdti_raw=np.stack([b for _, b in sdti_all]))


def _fold(a):
    """a [H, C] -> block-diag fold [H*C, H]."""
    H, C = a.shape
    out = np.zeros((H * C, H), np.float32)
    for h in range(H):
        out[h * C:(h + 1) * C, h] = a[h]
    return out


def _host_prep(inputs):
    x = np.asarray(inputs["x"], np.float32)
    W1 = np.asarray(inputs["W1"], np.float32)
    a1s = np.asarray(inputs["a1_src"], np.float32)
    a1d = np.asarray(inputs["a1_dst"], np.float32)
    W2 = np.asarray(inputs["W2"], np.float32)
    a2s = np.asarray(inputs["a2_src"], np.float32)
    a2d = np.asarray(inputs["a2_dst"], np.float32)
    Wc1 = np.asarray(inputs["Wc1"], np.float32)
    Wc2 = np.asarray(inputs["Wc2"], np.float32)

    shared = {}
    shared["xT"] = np.ascontiguousarray(x.T.reshape(2, 128, N)).astype(bf16)

    # Head-interleaved feature order: on-device hidden col j holds original
    # head-major dim perm1[j], i.e. j = c*H + h  <->  orig = h*C + c.  This
    # makes the per-head attention-weight broadcast step-1 on the innermost
    # axis so the DVE multiply runs in 2x mode.
    perm1 = np.array([(j % 8) * 16 + j // 8 for j in range(128)])   # j -> orig

    # L1 weights: w1cat[c] = [128, 288]; per type t: cols [t*144, t*144+128)
    # = W1[t] (cols permuted) rows 128c..128c+128, [+128, +136) = W1@fold(a1s).
    w1cat = np.zeros((2, 128, 288), np.float32)
    w1dst = np.zeros((2, 128, 16), np.float32)
    for t in range(2):
        ws = W1[t] @ _fold(a1s[t])          # [256, 8]
        wd = W1[t] @ _fold(a1d[t])          # [256, 8]
        W1p = W1[t][:, perm1]
        for c in range(2):
            w1cat[c, :, t * 144:t * 144 + 128] = W1p[c * 128:(c + 1) * 128]
            w1cat[c, :, t * 144 + 128:t * 144 + 136] = ws[c * 128:(c + 1) * 128]
            w1dst[c, :, t * 8:(t + 1) * 8] = wd[c * 128:(c + 1) * 128]
    shared["w1cat"] = w1cat.astype(bf16)
    shared["w1dst"] = w1dst.astype(bf16)

    # L2 weights: rows permuted by perm1 (h_mid cols are permuted), msg cols
    # permuted by perm2 (c*H + h).  w2cat [128, 1040]: [0:512] W2a, [512:1024]
    # W2b, [1024:1032] a-ssrc fold, [1032:1040] b-ssrc fold.  w2dst [128, 16].
    perm2 = np.array([(j % 8) * 64 + j // 8 for j in range(512)])
    w2cat = np.zeros((128, 1040), np.float32)
    w2dst = np.zeros((128, 16), np.float32)
    for t in range(2):
        w2cat[:, t * 512:(t + 1) * 512] = W2[t][perm1][:, perm2]
        w2cat[:, 1024 + t * 8:1032 + t * 8] = (W2[t] @ _fold(a2s[t]))[perm1]
        w2dst[:, t * 8:(t + 1) * 8] = (W2[t] @ _fold(a2d[t]))[perm1]
    shared["w2cat"] = w2cat.astype(bf16)
    shared["w2dst"] = w2dst.astype(bf16)

    shared["wc1"] = Wc1.astype(bf16)
    shared["wc2"] = Wc2.astype(bf16)

    S = np.zeros((128, GD), np.float32)
    for e in range(128):
        S[e, e % GD] = 1.0
    shared["sconst"] = S.astype(bf16)
    shared["ident"] = np.eye(128, dtype=np.float32).astype(bf16)
    p1 = np.zeros((1, T1C), np.float32)
    p1[0, 128:136] = -600.0
    shared["poison1"] = p1.astype(bf16)
    p2 = np.zeros((1, T2C), np.float32)
    p2[0, 512:520] = -600.0
    shared["poison2"] = p2.astype(bf16)

    ei_a = np.asarray(inputs["edge_index_a"])
    ei_b = np.asarray(inputs["edge_index_b"])
    sched_a = _schedule_for_type(ei_a[0], ei_a[1])
    sched_b = _schedule_for_type(ei_b[0], ei_b[1])

    per_core = []
    for k in range(NCORES):
        m = dict(shared)
        xds = np.zeros((2, 2, 128, 2 * NSLOT), np.float32)
        for t, sch in ((0, sched_a), (1, sched_b)):
            dd = sch["sdti_raw"][k]
            rows = np.where(dd == ND, k * ND, k * ND + dd)
            m_t = x[rows].T                      # [256, 2*NSLOT]
            xds[t] = m_t.reshape(2, 128, 2 * NSLOT)
        m["xds"] = xds.astype(bf16)
        m["gidx_a"] = sched_a["gidx"][k]
        m["gidx_b"] = sched_b["gidx"][k]
        m["scat_a"] = sched_a["scat"][k]
        m["scat_b"] = sched_b["scat"][k]
        m["sdti_a"] = sched_a["sdti"][k]
        m["sdti_b"] = sched_b["sdti"][k]
        per_core.append(m)
    return per_core, sched_a, sched_b


# ----------------------------------------------------------------------------
# device kernel
# ----------------------------------------------------------------------------

def _patch_queue_aware_lanes():
    """Make Tile's SWDGE DMA semaphore-lane assignment queue-aware: queue q
    gets lanes {2q, 2q+1}."""
    import concourse.tile_sem_assignment as tsa
    import concourse.mybir as mybir
    if getattr(tsa, "_qaware_patched", False):
        return
    orig = tsa.TileClockTick._assign_tick

    def patched(self, inst):
        if (isinstance(inst, tsa.DMAInst)
                and inst.engine == mybir.EngineType.Pool
                and not isinstance(inst, tsa.bass_isa.UserSyncedRemoteDMADescs)):
            q = getattr(inst, "queue_num", 0) or 0
            tog = getattr(self, "_q_toggle", None)
            if tog is None:
                tog = self._q_toggle = {}
            self.next_sw_dma_idx = (q * 2 + tog.get(q, 0)) % self.swdge_sem_count
            tog[q] = 1 - tog.get(q, 0)
        return orig(self, inst)

    tsa.TileClockTick._assign_tick = patched
    tsa._qaware_patched = True


def _build_nc(sched_a, sched_b):
    import concourse.bacc as bacc
    import concourse.mybir as mybir
    import concourse.tile as tile

    _patch_queue_aware_lanes()

    BF = mybir.dt.bfloat16
    F32 = mybir.dt.float32
    F8 = mybir.dt.float8e4
    I16 = mybir.dt.int16
    AF = mybir.ActivationFunctionType
    OP = mybir.AluOpType
    AX = mybir.AxisListType

    nc = bacc.Bacc("TRN2", target_bir_lowering=False, debug=False,
                   num_devices=NCORES, num_swdge_queues=4)

    def din(name, shape, dt=BF):
        return nc.dram_tensor(name, shape, dt, kind="ExternalInput").ap()

    scheds = {"a": sched_a, "b": sched_b}
    NSC = NSLOT // 16
    NCC = NSLOT // 128

    xT = din("xT", [2, 128, N])
    xds = din("xds", [2, 2, 128, 2 * NSLOT])
    w1cat = din("w1cat", [2, 128, 288])
    w1dst = din("w1dst", [2, 128, 16])
    w2cat = din("w2cat", [128, 1040])
    w2dst = din("w2dst", [128, 16])
    wc1 = din("wc1", [64, 32])
    wc2 = din("wc2", [32, 2])
    sconst = din("sconst", [128, GD])
    ident = din("ident", [128, 128])
    poison1 = din("poison1", [1, T1C])
    poison2 = din("poison2", [1, T2C])
    gidx_d = {t: din(f"gidx_{t}", [scheds[t]["TCpad"] // NCH, 128, NCH * 8], I16)
              for t in "ab"}
    scat_d = {t: din(f"scat_{t}", [128, NSC], I16) for t in "ab"}
    sdti_d = {t: din(f"sdti_{t}", [128, 2 * NSC], I16) for t in "ab"}
    out = nc.dram_tensor("out", [ND, 2], F32, kind="ExternalOutput").ap()

    qrr_state = {"a": 0, "b": 0, None: 0}

    def qrr(t=None):
        i = qrr_state[t]
        qrr_state[t] = 1 - i
        if t is None:
            q = qrr_state["__g"] = (qrr_state.get("__g", -1) + 1) % 4
            return q
        return (0 if t == "a" else 2) + i

    with tile.TileContext(nc) as tc:
        with tc.tile_pool(name="dram", bufs=1, space="DRAM") as dpool, \
             tc.tile_pool(name="const", bufs=1) as cpool:

            table1cat = dpool.tile([2, N + 1, T1C], BF, tag="tb1")
            table2cat = dpool.tile([2, N + 1, T2C], BF, tag="tb2")
            # local per-core dst-score tables [ND+1, 128]: cols 0:16 scores
            sdtab2 = dpool.tile([ND + 1, 128], BF, tag="sdtab2")
            acc1 = dpool.tile([NSLOT, 128], F32, tag="acc1")
            acc2 = dpool.tile([NSLOT, 64], F32, tag="acc2")
            # AllGather chunking: 2 column chunks of <=1280 local nodes
            AGC = 1280
            NAG = 2
            h2sliceT = dpool.tile([NAG, 128, AGC], BF, tag="h2sT")
            h2fullT = dpool.tile([NAG, NCORES, 128, AGC], BF, tag="h2fT")

            # ---- constants ----
            sconst_sb = cpool.tile([128, GD], BF)
            nc.sync.dma_start(sconst_sb[:], sconst[:])
            id_sb = cpool.tile([128, 128], BF)
            nc.sync.dma_start(id_sb[:], ident[:])
            wc1_sb = cpool.tile([64, 32], BF)
            nc.sync.dma_start(wc1_sb[:], wc1[:])
            wc2_sb = cpool.tile([32, 2], BF)
            nc.sync.dma_start(wc2_sb[:], wc2[:])
            w1cat_sb = cpool.tile([128, 2, 288], BF)
            nc.sync.dma_start(w1cat_sb[:], w1cat.rearrange("c p f -> p c f"))
            w1dst_sb = cpool.tile([128, 2, 16], BF)
            nc.sync.dma_start(w1dst_sb[:], w1dst.rearrange("c p f -> p c f"))
            w2cat_sb = cpool.tile([128, 1040], BF)
            nc.sync.dma_start(w2cat_sb[:], w2cat[:])
            w2dst_sb = cpool.tile([128, 16], BF)
            nc.sync.dma_start(w2dst_sb[:], w2dst[:])
            scat_sb = {}
            sdti_sb = {}
            for t in "ab":
                scat_sb[t] = cpool.tile([128, NSC], I16, tag=f"scat{t}", name=f"scatsb{t}")
                nc.sync.dma_start(scat_sb[t][:], scat_d[t][:])
                sdti_sb[t] = cpool.tile([128, 2 * NSC], I16, tag=f"sdti{t}", name=f"sdtisb{t}")
                nc.sync.dma_start(sdti_sb[t][:], sdti_d[t][:])
            gidx_sb = {}
            for t in "ab":
                ncalls = scheds[t]["TCpad"] // NCH
                gidx_sb[t] = cpool.tile([128, ncalls, NCH * 8], I16,
                                        tag=f"gidx{t}", name=f"gidxsb{t}")
                nc.scalar.dma_start(gidx_sb[t][:],
                                    gidx_d[t].rearrange("c p s -> p c s"))

            for ti in range(2):
                nc.sync.dma_start(table1cat[ti, N:N + 1, :], poison1[:])
                nc.sync.dma_start(table2cat[ti, N:N + 1, :], poison2[:])

            NT = (ND + 127) // 128    # 20 tiles of local dst rows

            # ---- zero accumulators + sdtab poison rows ----
            with tc.tile_pool(name="zacc", bufs=1) as zaccp:
                zt = zaccp.tile([128, NCC, 128], F32)
                nc.vector.memset(zt[:], 0.0)
                nc.sync.dma_start(acc1.rearrange("(a p) c -> p a c", p=128), zt[:])
                nc.sync.dma_start(acc2.rearrange("(a p) c -> p a c", p=128),
                                  zt[:, :, 0:64])
                zb = zaccp.tile([1, 128], BF)
                nc.vector.memset(zb[:], 0.0)
                nc.sync.dma_start(sdtab2[ND:ND + 1, :], zb[:])

            # ---- sdt1s: slot-ordered dst scores for L1, computed straight
            # into SBUF (no DRAM round trip, no gathers) ----
            sdt1s = cpool.tile([128, NG, 16], BF, tag="sdt1s")
            with tc.tile_pool(name="sd1", bufs=3) as sd1p, \
                 tc.tile_pool(name="sd1ps", bufs=2, space="PSUM") as sd1ps:
                xds_sb = sd1p.tile([128, 2, 2, 2 * NSLOT], BF, tag="xd", bufs=1)
                nc.scalar.dma_start(xds_sb[:],
                                    xds.rearrange("t c p s -> p t c s"))
                for i in range(2 * NSLOT // 128):
                    lo = i * 128
                    for ti in range(2):
                        ps = sd1ps.tile([128, 8], F32, tag="sd1ps")
                        nc.tensor.matmul(
                            out=ps[:], lhsT=xds_sb[:, ti, 0, lo:lo + 128],
                            rhs=w1dst_sb[:, 0, ti * 8:ti * 8 + 8],
                            start=True, stop=False)
                        nc.tensor.matmul(
                            out=ps[:], lhsT=xds_sb[:, ti, 1, lo:lo + 128],
                            rhs=w1dst_sb[:, 1, ti * 8:ti * 8 + 8],
                            start=False, stop=True)
                        nc.vector.tensor_copy(
                            out=sdt1s[:, i, ti * 8:ti * 8 + 8], in_=ps[:])

            # ---- phase 1: layer-1 tables (types fused) ----
            with tc.tile_pool(name="ph1", bufs=4) as p1p, \
                 tc.tile_pool(name="ph1ps", bufs=3, space="PSUM") as p1ps:
                xt_sb = [p1p.tile([128, N], BF, tag=f"xt{c}", name=f"xtsb{c}", bufs=1) for c in range(2)]
                nc.scalar.dma_start(xt_sb[0][:], xT[0])
                nc.sync.dma_start(xt_sb[1][:], xT[1])
                for i in range((N + 127) // 128):
                    lo = i * 128
                    m = min(128, N - lo)
                    ps = p1ps.tile([128, 288], F32, tag="t1ps")
                    nc.tensor.matmul(out=ps[:m], lhsT=xt_sb[0][:, lo:lo + m],
                                     rhs=w1cat_sb[:, 0, :], start=True, stop=False)
                    nc.tensor.matmul(out=ps[:m], lhsT=xt_sb[1][:, lo:lo + m],
                                     rhs=w1cat_sb[:, 1, :], start=False, stop=True)
                    o = p1p.tile([128, 288], BF, tag="t1o")
                    if i % 2 == 0:
                        nc.scalar.copy(out=o[:m], in_=ps[:m])
                    else:
                        nc.vector.tensor_copy(out=o[:m], in_=ps[:m])
                    eng = nc.sync if i % 2 == 0 else nc.scalar
                    eng.dma_start(
                        table1cat[:, lo:lo + m, 0:144].rearrange(
                            "t p f -> p t f"),
                        o[:m].rearrange("p (t f) -> p t f", t=2))

            # ---- edge phase ----
            def gather_sdt(layer, pool, sdtab):
                sdt = {}
                for t in "ab":
                    sdt[t] = pool.tile([128, NG, 128], BF, tag=f"sdt{t}",
                                       name=f"sdt{layer}{t}", bufs=1)
                    sdone = 0
                    while sdone < 2 * NSLOT:
                        n = min(1024, 2 * NSLOT - sdone)
                        nc.gpsimd.dma_gather(
                            sdt[t][:, sdone // 128:(sdone + n) // 128, :],
                            sdtab[:],
                            sdti_sb[t][:, sdone // 16:(sdone + n) // 16],
                            n, n, 128, elem_step=128, queue_num=qrr(t))
                        sdone += n
                return sdt

            def edge_phase(layer, tables, parks, sdt, acc, gbufs):
                CT = T1C if layer == 1 else T2C
                C = 128 if layer == 1 else 512
                SC = C
                hb = C // 8
                NV = 128 // GD

                with tc.tile_pool(name=f"eg{layer}", bufs=gbufs) as gp, \
                     tc.tile_pool(name=f"ew{layer}", bufs=4) as wp, \
                     tc.tile_pool(name=f"es{layer}", bufs=6) as sp, \
                     tc.tile_pool(name=f"ef{layer}", bufs=2) as fp, \
                     tc.tile_pool(name=f"eps{layer}", bufs=2, space="PSUM") as pp, \
                     tc.tile_pool(name=f"ezs{layer}", bufs=2, space="PSUM") as zp:

                    st = {t: dict(call=-1, G=None, Wb=None, pa=None, pz=None)
                          for t in "ab"}

                    def do_group(t, g):
                        ti = 0 if t == "a" else 1
                        sched = scheds[t]
                        cg = int(sched["Rg"][g] // (128 // GD))
                        base = int(sched["cbase"][g])
                        s_ = st[t]
                        if g % NV == 0:
                            s_["pa"] = pp.tile([128, C], F32, tag=f"pa{t}",
                                               name=f"pa{layer}{t}")
                            s_["pz"] = zp.tile([128, NCH, 8], F32, tag=f"pz{t}",
                                               name=f"pz{layer}{t}")
                            nc.vector.memset(s_["pz"][:], 0.0)
                        pa, pz = s_["pa"], s_["pz"]
                        row0 = GD * (g % NV)
                        done = 0
                        while done < cg:
                            seg = min(NCH - (base + done) % NCH, cg - done)
                            call = (base + done) // NCH
                            coff = (base + done) % NCH
                            if call != s_["call"]:
                                G = gp.tile([128, NCH, CT], BF, tag=f"G{t}",
                                            name=f"G{layer}{t}")
                                nc.gpsimd.dma_gather(
                                    G[:, :, :], tables[t][:],
                                    gidx_sb[t][:, call, :],
                                    NCH * 128, NCH * 128, CT,
                                    queue_num=qrr(t))
                                s_["call"] = call
                                s_["G"] = G
                            G = s_["G"]
                            M = G
                            sl = slice(coff, coff + seg)
                            sview = G[:, sl, SC:SC + 8]
                            u = sp.tile([128, NCH, 8], F32, tag=f"u{t}",
                                        name=f"u{layer}{t}")
                            nc.vector.tensor_tensor(
                                out=u[:, :seg, :], in0=sview,
                                in1=sdt[t][:, g, ti * 8:ti * 8 + 8][:, None, :]
                                    .to_broadcast([128, seg, 8]),
                                op=OP.add)
                            phi = sp.tile([128, NCH, 8], F32, tag=f"phi{t}",
                                          name=f"phi{layer}{t}")
                            nc.vector.scalar_tensor_tensor(
                                out=phi[:, :seg, :], in0=u[:, :seg, :], scalar=NEG,
                                in1=u[:, :seg, :], op0=OP.mult, op1=OP.max)
                            q = sp.tile([128, NCH, 8], BF, tag=f"q{t}",
                                        name=f"q{layer}{t}")
                            nc.scalar.activation(out=q[:, :seg, :],
                                                 in_=phi[:, :seg, :], func=AF.Exp)
                            nc.vector.tensor_tensor(
                                out=M[:, sl, 0:C].rearrange(
                                    "p s (c h) -> p s c h", h=8),
                                in0=M[:, sl, 0:C].rearrange(
                                    "p s (c h) -> p s c h", h=8),
                                in1=q[:, :seg, None, :].to_broadcast(
                                    [128, seg, hb, 8]),
                                op=OP.mult)
                            nc.tensor.matmul(
                                out=pz[row0:row0 + GD, :seg, :],
                                lhsT=sconst_sb[:], rhs=q[:, :seg, :],
                                start=False, stop=(done + seg == cg),
                                skip_group_check=True)
                            for s in range(seg):
                                cc = done + s
                                nc.tensor.matmul(
                                    out=pa[row0:row0 + GD, :],
                                    lhsT=sconst_sb[:], rhs=M[:, coff + s, 0:C],
                                    start=(cc == 0), stop=(cc == cg - 1),
                                    skip_group_check=True)
                            done += seg
                        if g % NV == NV - 1:
                            mi = (g * GD) // 128
                            zs = sp.tile([128, 8], F32, tag=f"zs{t}",
                                         name=f"zs{layer}{t}")
                            nc.vector.tensor_reduce(
                                out=zs[:, :, None],
                                in_=pz.rearrange("p s h -> p h s"),
                                axis=AX.X, op=OP.add)
                            z8 = sp.tile([128, 8], F32, tag=f"z8{t}",
                                         name=f"z8{layer}{t}")
                            nc.vector.tensor_scalar(
                                out=z8[:], in0=zs[:],
                                scalar1=(1.0 if layer == 1 else 8.0),
                                scalar2=1e-30, op0=OP.mult, op1=OP.max)
                            rz = sp.tile([128, 8], F32, tag=f"rz{t}",
                                         name=f"rz{layer}{t}")
                            nc.vector.reciprocal(out=rz[:], in_=z8[:])
                            if layer == 1:
                                nc.vector.tensor_tensor(
                                    out=parks[t][:, mi, :].rearrange(
                                        "p (c h) -> p c h", h=8),
                                    in0=pa[:, 0:128].rearrange(
                                        "p (c h) -> p c h", h=8),
                                    in1=rz[:, None, :].to_broadcast(
                                        [128, 16, 8]),
                                    op=OP.mult)
                            else:
                                tmp = fp.tile([128, 512], F32, tag=f"tmp{t}",
                                              name=f"tmp{layer}{t}")
                                nc.vector.tensor_tensor(
                                    out=tmp[:].rearrange("p (c h) -> p c h", h=8),
                                    in0=pa[:].rearrange("p (c h) -> p c h", h=8),
                                    in1=rz[:, None, :].to_broadcast(
                                        [128, 64, 8]),
                                    op=OP.mult)
                                nc.vector.tensor_reduce(
                                    out=parks[t][:, mi, :, None],
                                    in_=tmp[:].rearrange("p (c h) -> p c h", h=8),
                                    axis=AX.X, op=OP.add)

                    CA = 128 if layer == 1 else 64
                    windows = {15: (0, 8), 31: (8, 8), 39: (16, 4)}
                    for g in range(NG):
                        for t in "ab":
                            do_group(t, g)
                        if g in windows:
                            c0, nb = windows[g]
                            for t in "ab":
                                nc.gpsimd.dma_scatter_add(
                                    acc[:], parks[t][:, c0:c0 + nb, :],
                                    scat_sb[t][:, c0 * 8:(c0 + nb) * 8],
                                    nb * 128, nb * 128, CA, queue_num=qrr(t))

            # ---- layer-1 edges ----
            with tc.tile_pool(name="park1", bufs=1) as parkp:
                parks = {t: parkp.tile([128, NCC, 128], F32, tag=f"park{t}",
                                       name=f"park1{t}") for t in "ab"}
                edge_phase(1, {"a": table1cat[0], "b": table1cat[1]}, parks,
                           {"a": sdt1s, "b": sdt1s}, acc1, gbufs=8)

            # ---- combine + ELU helper ----
            def elu_combine(src_ap, cols, tilepool, dst_write):
                for i in range(NT):
                    lo = i * 128
                    m = min(128, ND - lo)
                    a = tilepool.tile([128, cols], F32, tag="ec_a")
                    nc.sync.dma_start(a[:m], src_ap[lo:lo + m, :])
                    e = tilepool.tile([128, cols], F32, tag="ec_e")
                    nc.scalar.activation(out=e[:m], in_=a[:m], func=AF.Exp, scale=0.5)
                    em1 = tilepool.tile([128, cols], F32, tag="ec_em1")
                    nc.vector.tensor_scalar(out=em1[:m], in0=e[:m], scalar1=-1.0,
                                            scalar2=None, op0=OP.add)
                    xm = tilepool.tile([128, cols], F32, tag="ec_xm")
                    nc.vector.tensor_scalar(out=xm[:m], in0=a[:m], scalar1=0.5,
                                            scalar2=None, op0=OP.mult)
                    mk = tilepool.tile([128, cols], mybir.dt.uint8, tag="ec_mk")
                    nc.vector.tensor_scalar(out=mk[:m], in0=a[:m], scalar1=0.0,
                                            scalar2=None, op0=OP.is_gt)
                    h = tilepool.tile([128, cols], BF, tag="ec_h")
                    nc.vector.select(out=h[:m], mask=mk[:m], on_true=xm[:m],
                                     on_false=em1[:m])
                    dst_write(i, lo, m, h)

            # L1 combine -> transposed slice; also L2 dst scores -> sdtab2;
            # AllGather fires per finished column chunk.
            with tc.tile_pool(name="elu1", bufs=4) as elup, \
                 tc.tile_pool(name="elu1ps", bufs=3, space="PSUM") as elups:
                def wr1(i, lo, m, h):
                    c = i // 10
                    off = (i % 10) * 128
                    tps = elups.tile([128, 128], BF, tag="e_tp")
                    nc.tensor.transpose(out=tps[:, :m], in_=h[:m, :],
                                        identity=id_sb[:m, :m])
                    ht = elup.tile([128, 128], BF, tag="e_ht")
                    nc.scalar.copy(out=ht[:, :m], in_=tps[:, :m])
                    nc.scalar.dma_start(h2sliceT[c, :, off:off + m], ht[:, :m])
                    ps2 = elups.tile([128, 16], F32, tag="e_sd2")
                    nc.tensor.matmul(out=ps2[:m], lhsT=ht[:, :m],
                                     rhs=w2dst_sb[:], start=True, stop=True)
                    o2 = elup.tile([128, 16], BF, tag="e_sd2o")
                    nc.vector.tensor_copy(out=o2[:m], in_=ps2[:m])
                    nc.scalar.dma_start(sdtab2[lo:lo + m, 0:16], o2[:m])
                    if i % 10 == 9 or i == NT - 1:
                        nc.gpsimd.collective_compute(
                            "AllGather", mybir.AluOpType.bypass,
                            replica_groups=[list(range(NCORES))],
                            ins=[h2sliceT[c].opt()], outs=[h2fullT[c].opt()])
                elu_combine(acc1[:, :], 128, elup, wr1)

            sdt2ctx = tc.tile_pool(name="sdt2p", bufs=1)
            sdt2p = sdt2ctx.__enter__()
            sdt2 = gather_sdt(2, sdt2p, sdtab2)

            # ---- phase 4: layer-2 tables from SBUF-resident h2T (fused) ----
            with tc.tile_pool(name="ph4", bufs=6) as p4p, \
                 tc.tile_pool(name="ph4ps", bufs=3, space="PSUM") as p4ps, \
                 tc.tile_pool(name="ph4px", bufs=2, space="PSUM") as p4px:
                for c in range(NAG):
                    h2t_sb = p4p.tile([128, NCORES, AGC], BF, tag="h2t",
                                      name="h2tsb", bufs=2)
                    nc.sync.dma_start(h2t_sb[:],
                                      h2fullT[c].rearrange("k p j -> p k j"))
                    ctiles = min(10, (ND - c * AGC + 127) // 128)
                    for k8 in range(NCORES):
                        for jj in range(ctiles):
                            off = jj * 128
                            m = min(128, ND - c * AGC - off)
                            row = k8 * ND + c * AGC + off
                            lhs = h2t_sb[:, k8, off:off + m]
                            psa = p4ps.tile([128, 512], F32, tag="t2psa")
                            nc.tensor.matmul(out=psa[:m], lhsT=lhs,
                                             rhs=w2cat_sb[:, 0:512],
                                             start=True, stop=True)
                            psb = p4ps.tile([128, 512], F32, tag="t2psb")
                            nc.tensor.matmul(out=psb[:m], lhsT=lhs,
                                             rhs=w2cat_sb[:, 512:1024],
                                             start=True, stop=True)
                            psx = p4px.tile([128, 16], F32, tag="t2psx")
                            nc.tensor.matmul(out=psx[:m], lhsT=lhs,
                                             rhs=w2cat_sb[:, 1024:1040],
                                             start=True, stop=True)
                            o2 = p4p.tile([128, 2, 520], BF, tag="t2o")
                            nc.scalar.copy(out=o2[:m, 0, 0:512], in_=psa[:m])
                            nc.vector.tensor_copy(out=o2[:m, 1, 0:512],
                                                  in_=psb[:m])
                            nc.scalar.copy(out=o2[:m, 0, 512:520],
                                           in_=psx[:m, 0:8])
                            nc.vector.tensor_copy(out=o2[:m, 1, 512:520],
                                                  in_=psx[:m, 8:16])
                            eng = nc.sync if jj % 2 == 0 else nc.scalar
                            eng.dma_start(
                                table2cat[:, row:row + m, 0:520].rearrange(
                                    "t p f -> p t f"),
                                o2[:m])

            # ---- layer-2 edges ----
            with tc.tile_pool(name="park2", bufs=1) as park2p:
                parks = {t: park2p.tile([128, NCC, 64], F32, tag=f"park2{t}",
                                        name=f"park2{t}") for t in "ab"}
                edge_phase(2, {"a": table2cat[0], "b": table2cat[1]}, parks,
                           sdt2, acc2, gbufs=6)
            sdt2ctx.__exit__(None, None, None)

            # ---- classifier ----
            with tc.tile_pool(name="cls", bufs=4) as clsp, \
                 tc.tile_pool(name="clsps", bufs=2, space="PSUM") as clsps:
                def wrc(i, lo, m, h):
                    tps = clsps.tile([64, 128], BF, tag="c_t1")
                    nc.tensor.transpose(out=tps[:, :m], in_=h[:m, :],
                                        identity=id_sb[:m, :m])
                    h3t = clsp.tile([64, 128], BF, tag="c_h3t")
                    nc.scalar.copy(out=h3t[:, :m], in_=tps[:, :m])
                    z1 = clsps.tile([128, 32], F32, tag="c_z1")
                    nc.tensor.matmul(out=z1[:m], lhsT=h3t[:, :m], rhs=wc1_sb[:],
                                     start=True, stop=True)
                    z1s = clsp.tile([128, 32], BF, tag="c_z1s")
                    nc.scalar.activation(out=z1s[:m], in_=z1[:m], func=AF.Relu)
                    t2ps = clsps.tile([32, 128], BF, tag="c_t2")
                    nc.tensor.transpose(out=t2ps[:, :m], in_=z1s[:m, :],
                                        identity=id_sb[:m, :m])
                    z1t = clsp.tile([32, 128], BF, tag="c_z1t")
                    nc.scalar.copy(out=z1t[:, :m], in_=t2ps[:, :m])
                    lg = clsps.tile([128, 2], F32, tag="c_lg")
                    nc.tensor.matmul(out=lg[:m], lhsT=z1t[:, :m], rhs=wc2_sb[:],
                                     start=True, stop=True)
                    lo_ = clsp.tile([128, 2], F32, tag="c_out")
                    nc.vector.tensor_copy(out=lo_[:m], in_=lg[:m])
                    nc.sync.dma_start(out[lo:lo + m, :], lo_[:m])
                elu_combine(acc2[:, :], 64, clsp, wrc)

    nc.compile()
    return nc


# ----------------------------------------------------------------------------
# entry point
# ----------------------------------------------------------------------------

_CACHE = {}


def _prepare(inputs):
    per_core, sched_a, sched_b = _host_prep(inputs)
    key = (sched_a["TCpad"], sched_b["TCpad"],
           tuple(sched_a["Rg"]), tuple(sched_b["Rg"]))
    if key not in _CACHE:
        _CACHE.clear()
        _CACHE[key] = _build_nc(sched_a, sched_b)
    return _CACHE[key], per_core


def _run(nc, per_core, **kw):
    from concourse import bass_utils
    return bass_utils.run_bass_kernel_spmd(nc, per_core,
                                           core_ids=list(range(NCORES)), **kw)


def kernel(**inputs):
    nc, per_core = _prepare(inputs)
    res = _run(nc, per_core)
    return np.concatenate([res.results[k]["out"] for k in range(NCORES)], 0)


# revision 53
# speedup vs baseline: 1.0404x; 1.0119x over previous
"""Trainium2 Bass kernel for nn_BotAwareGAT (2-layer hetero GAT + MLP).

Strategy (8 NeuronCores, SPMD):
  - Destination-partitioned: core k owns dst nodes [k*2500, (k+1)*2500).
  - Dense projections replicated; per-edge softmax-aggregation uses an ELL
    schedule: per edge type the core's dsts are degree-sorted and packed into
    groups of 64; each 128-edge chunk covers 2 rounds of a group so the
    segment-sum one-hot matrix is a compile-time constant.  Messages are
    fetched with batched dma_gather (4 SWDGE queues round-robin) from a
    node-major table [h | s_src]; attention weights q=exp(lrelu(...)) are
    built on DVE/ACT; the gathered messages are scaled by q in place; one PE
    matmul per chunk accumulates messages per dst, one per piece accumulates
    q; normalization happens per 128-slot batch.
  - Per-dst scores (s_dst) live in tiny local tables built from per-core
    sliced inputs, so score-distribution gathers overlap the table builds.
  - All weight augmentation (attention-vector folds) precomputed on host.
  - Layer-1 results are exchanged with AllGather; padding edges point at a
    poison table row whose s_src = -600 so exp() underflows to 0.
"""

import numpy as np
import ml_dtypes

N = 20000
NCORES = 8
ND = N // NCORES            # 2500 dst nodes per core
GD = 64                     # dsts per group
NSLOT = 2560                # padded slots per core (40 groups)
NG = NSLOT // GD            # 40 groups
HEADS = 8
POISON = N                  # poison row index in big tables
T1C = 256                   # L1 table cols (bf16): [h1(128) | ssrc(8) | pad]
T2C = 640                   # L2 table cols: [h2(512) | ssrc(8) | pad]
NCH = 8                     # max chunks per dma_gather call (1024 idx)
NEG = 0.2

bf16 = ml_dtypes.bfloat16


# ----------------------------------------------------------------------------
# host-side schedule construction
# ----------------------------------------------------------------------------

def _wrap16(a):
    """[L] int -> [128, L//16] int16 (dma_gather/scatter index layout,
    replicated over the 8 q7 partition groups)."""
    w = a.reshape(-1, 16).T.astype(np.int16)
    return np.tile(w, (8, 1))


def _schedule_for_type(src, dst):
    """Build per-core ELL schedules for one edge type."""
    percore = []
    for k in range(NCORES):
        sel = (dst >= k * ND) & (dst < (k + 1) * ND)
        s = src[sel].astype(np.int64)
        d = (dst[sel] - k * ND).astype(np.int64)
        deg = np.bincount(d, minlength=ND)
        order = np.argsort(-deg, kind="stable")
        o = np.argsort(d, kind="stable")
        s_sorted = s[o]
        starts = np.zeros(ND + 1, np.int64)
        np.cumsum(deg, out=starts[1:])
        percore.append((deg, order, s_sorted, starts))

    Rg = np.zeros(NG, np.int64)
    for g in range(NG):
        mx = 1
        lo, hi = g * GD, (g + 1) * GD
        for (deg, order, _, _) in percore:
            real_hi = min(hi, ND)
            if lo < real_hi:
                mx = max(mx, int(deg[order[lo:real_hi]].max()))
        rpc = 128 // GD
        Rg[g] = ((mx + rpc - 1) // rpc) * rpc
    Cg = Rg // (128 // GD)
    cbase = np.zeros(NG, np.int64)
    np.cumsum(Cg[:-1], out=cbase[1:])
    TC = int(Cg.sum())
    TCpad = ((TC + NCH - 1) // NCH) * NCH

    gidx_all, scat_all, sdti_all = [], [], []
    for k in range(NCORES):
        deg, order, s_sorted, starts = percore[k]
        gidx = np.full(TCpad * 128, POISON, np.int64)
        scat = np.zeros(NSLOT, np.int64)
        sdti = np.full(2 * NSLOT, ND, np.int64)   # local poison row = ND
        for g in range(NG):
            base = cbase[g]
            for j in range(GD):
                rank = g * GD + j
                if rank < ND:
                    dd = int(order[rank])
                    scat[rank] = dd
                    sdti[g * 128 + j] = dd
                    sdti[g * 128 + GD + j] = dd
                    dg = int(deg[dd])
                    if dg:
                        r = np.arange(dg)
                        rpc = 128 // GD
                        pos = (base + r // rpc) * 128 + (r % rpc) * GD + j
                        gidx[pos] = s_sorted[starts[dd]:starts[dd] + dg]
                else:
                    scat[rank] = rank   # trash rows 2500..2559
        ncalls = TCpad // NCH
        gidx_all.append(_wrap16(gidx).reshape(128, ncalls, NCH * 8).transpose(1, 0, 2))
        scat_all.append(_wrap16(scat))
        sdti_all.append((_wrap16(sdti), sdti.copy()))

    return dict(Rg=Rg, cbase=cbase, TC=TC, TCpad=TCpad,
                gidx=np.stack(gidx_all), scat=np.stack(scat_all),
                sdti=np.stack([a for a, _ in sdti_all]),
                sdti_raw=np.stack([b for _, b in sdti_all]))


def _fold(a):
    """a [H, C] -> block-diag fold [H*C, H]."""
    H, C = a.shape
    out = np.zeros((H * C, H), np.float32)
    for h in range(H):
        out[h * C:(h + 1) * C, h] = a[h]
    return out


def _host_prep(inputs):
    x = np.asarray(inputs["x"], np.float32)
    W1 = np.asarray(inputs["W1"], np.float32)
    a1s = np.asarray(inputs["a1_src"], np.float32)
    a1d = np.asarray(inputs["a1_dst"], np.float32)
    W2 = np.asarray(inputs["W2"], np.float32)
    a2s = np.asarray(inputs["a2_src"], np.float32)
    a2d = np.asarray(inputs["a2_dst"], np.float32)
    Wc1 = np.asarray(inputs["Wc1"], np.float32)
    Wc2 = np.asarray(inputs["Wc2"], np.float32)

    shared = {}
    shared["xT"] = np.ascontiguousarray(x.T.reshape(2, 128, N)).astype(bf16)

    # Head-interleaved feature order: on-device hidden col j holds original
    # head-major dim perm1[j], i.e. j = c*H + h  <->  orig = h*C + c.  This
    # makes the per-head attention-weight broadcast step-1 on the innermost
    # axis so the DVE multiply runs in 2x mode.
    perm1 = np.array([(j % 8) * 16 + j // 8 for j in range(128)])   # j -> orig

    # L1 weights: w1cat[c] = [128, 288]; per type t: cols [t*144, t*144+128)
    # = W1[t] (cols permuted) rows 128c..128c+128, [+128, +136) = W1@fold(a1s).
    w1cat = np.zeros((2, 128, 288), np.float32)
    w1dst = np.zeros((2, 128, 16), np.float32)
    for t in range(2):
        ws = W1[t] @ _fold(a1s[t])          # [256, 8]
        wd = W1[t] @ _fold(a1d[t])          # [256, 8]
        W1p = W1[t][:, perm1]
        for c in range(2):
            w1cat[c, :, t * 144:t * 144 + 128] = W1p[c * 128:(c + 1) * 128]
            w1cat[c, :, t * 144 + 128:t * 144 + 136] = ws[c * 128:(c + 1) * 128]
            w1dst[c, :, t * 8:(t + 1) * 8] = wd[c * 128:(c + 1) * 128]
    shared["w1cat"] = w1cat.astype(bf16)
    shared["w1dst"] = w1dst.astype(bf16)

    # L2 weights: rows permuted by perm1 (h_mid cols are permuted), msg cols
    # permuted by perm2 (c*H + h).  w2cat [128, 1040]: [0:512] W2a, [512:1024]
    # W2b, [1024:1032] a-ssrc fold, [1032:1040] b-ssrc fold.  w2dst [128, 16].
    perm2 = np.array([(j % 8) * 64 + j // 8 for j in range(512)])
    w2cat = np.zeros((128, 1040), np.float32)
    w2dst = np.zeros((128, 16), np.float32)
    for t in range(2):
        w2cat[:, t * 512:(t + 1) * 512] = W2[t][perm1][:, perm2]
        w2cat[:, 1024 + t * 8:1032 + t * 8] = (W2[t] @ _fold(a2s[t]))[perm1]
        w2dst[:, t * 8:(t + 1) * 8] = (W2[t] @ _fold(a2d[t]))[perm1]
    shared["w2cat"] = w2cat.astype(bf16)
    shared["w2dst"] = w2dst.astype(bf16)

    shared["wc1"] = Wc1.astype(bf16)
    shared["wc2"] = Wc2.astype(bf16)

    S = np.zeros((128, GD), np.float32)
    for e in range(128):
        S[e, e % GD] = 1.0
    shared["sconst"] = S.astype(bf16)
    shared["ident"] = np.eye(128, dtype=np.float32).astype(bf16)
    p1 = np.zeros((1, T1C), np.float32)
    p1[0, 128:136] = -600.0
    shared["poison1"] = p1.astype(bf16)
    p2 = np.zeros((1, T2C), np.float32)
    p2[0, 512:520] = -600.0
    shared["poison2"] = p2.astype(bf16)

    ei_a = np.asarray(inputs["edge_index_a"])
    ei_b = np.asarray(inputs["edge_index_b"])
    sched_a = _schedule_for_type(ei_a[0], ei_a[1])
    sched_b = _schedule_for_type(ei_b[0], ei_b[1])

    per_core = []
    for k in range(NCORES):
        m = dict(shared)
        xds = np.zeros((2, 2, 128, 2 * NSLOT), np.float32)
        for t, sch in ((0, sched_a), (1, sched_b)):
            dd = sch["sdti_raw"][k]
            rows = np.where(dd == ND, k * ND, k * ND + dd)
            m_t = x[rows].T                      # [256, 2*NSLOT]
            xds[t] = m_t.reshape(2, 128, 2 * NSLOT)
        m["xds"] = xds.astype(bf16)
        m["gidx_a"] = sched_a["gidx"][k]
        m["gidx_b"] = sched_b["gidx"][k]
        m["scat_a"] = sched_a["scat"][k]
        m["scat_b"] = sched_b["scat"][k]
        m["sdti_a"] = sched_a["sdti"][k]
        m["sdti_b"] = sched_b["sdti"][k]
        per_core.append(m)
    return per_core, sched_a, sched_b


# ----------------------------------------------------------------------------
# device kernel
# ----------------------------------------------------------------------------

def _patch_queue_aware_lanes():
    """Make Tile's SWDGE DMA semaphore-lane assignment queue-aware: queue q
    gets lanes {2q, 2q+1}."""
    import concourse.tile_sem_assignment as tsa
    import concourse.mybir as mybir
    if getattr(tsa, "_qaware_patched", False):
        return
    orig = tsa.TileClockTick._assign_tick

    def patched(self, inst):
        if (isinstance(inst, tsa.DMAInst)
                and inst.engine == mybir.EngineType.Pool
                and not isinstance(inst, tsa.bass_isa.UserSyncedRemoteDMADescs)):
            q = getattr(inst, "queue_num", 0) or 0
            tog = getattr(self, "_q_toggle", None)
            if tog is None:
                tog = self._q_toggle = {}
            self.next_sw_dma_idx = (q * 2 + tog.get(q, 0)) % self.swdge_sem_count
            tog[q] = 1 - tog.get(q, 0)
        return orig(self, inst)

    tsa.TileClockTick._assign_tick = patched
    tsa._qaware_patched = True


def _build_nc(sched_a, sched_b):
    import concourse.bacc as bacc
    import concourse.mybir as mybir
    import concourse.tile as tile

    _patch_queue_aware_lanes()

    BF = mybir.dt.bfloat16
    F32 = mybir.dt.float32
    F8 = mybir.dt.float8e4
    I16 = mybir.dt.int16
    AF = mybir.ActivationFunctionType
    OP = mybir.AluOpType
    AX = mybir.AxisListType

    nc = bacc.Bacc("TRN2", target_bir_lowering=False, debug=False,
                   num_devices=NCORES, num_swdge_queues=4)

    def din(name, shape, dt=BF):
        return nc.dram_tensor(name, shape, dt, kind="ExternalInput").ap()

    scheds = {"a": sched_a, "b": sched_b}
    NSC = NSLOT // 16
    NCC = NSLOT // 128

    xT = din("xT", [2, 128, N])
    xds = din("xds", [2, 2, 128, 2 * NSLOT])
    w1cat = din("w1cat", [2, 128, 288])
    w1dst = din("w1dst", [2, 128, 16])
    w2cat = din("w2cat", [128, 1040])
    w2dst = din("w2dst", [128, 16])
    wc1 = din("wc1", [64, 32])
    wc2 = din("wc2", [32, 2])
    sconst = din("sconst", [128, GD])
    ident = din("ident", [128, 128])
    poison1 = din("poison1", [1, T1C])
    poison2 = din("poison2", [1, T2C])
    gidx_d = {t: din(f"gidx_{t}", [scheds[t]["TCpad"] // NCH, 128, NCH * 8], I16)
              for t in "ab"}
    scat_d = {t: din(f"scat_{t}", [128, NSC], I16) for t in "ab"}
    sdti_d = {t: din(f"sdti_{t}", [128, 2 * NSC], I16) for t in "ab"}
    out = nc.dram_tensor("out", [ND, 2], F32, kind="ExternalOutput").ap()

    qrr_state = {"a": 0, "b": 0, None: 0}

    def qrr(t=None):
        i = qrr_state[t]
        qrr_state[t] = 1 - i
        if t is None:
            q = qrr_state["__g"] = (qrr_state.get("__g", -1) + 1) % 4
            return q
        return (0 if t == "a" else 2) + i

    with tile.TileContext(nc) as tc:
        with tc.tile_pool(name="dram", bufs=1, space="DRAM") as dpool, \
             tc.tile_pool(name="const", bufs=1) as cpool:

            table1cat = dpool.tile([2, N + 1, T1C], BF, tag="tb1")
            table2cat = dpool.tile([2, N + 1, T2C], BF, tag="tb2")
            # local per-core dst-score tables [ND+1, 128]: cols 0:16 scores
            sdtab2 = dpool.tile([ND + 1, 128], BF, tag="sdtab2")
            acc1 = dpool.tile([NSLOT, 128], F32, tag="acc1")
            acc2 = dpool.tile([NSLOT, 64], F32, tag="acc2")
            # AllGather chunking: 2 column chunks of <=1280 local nodes
            AGC = 1280
            NAG = 2
            h2sliceT = dpool.tile([NAG, 128, AGC], BF, tag="h2sT")
            h2fullT = dpool.tile([NAG, NCORES, 128, AGC], BF, tag="h2fT")

            # ---- constants ----
            sconst_sb = cpool.tile([128, GD], BF)
            nc.sync.dma_start(sconst_sb[:], sconst[:])
            id_sb = cpool.tile([128, 128], BF)
            nc.sync.dma_start(id_sb[:], ident[:])
            wc1_sb = cpool.tile([64, 32], BF)
            nc.sync.dma_start(wc1_sb[:], wc1[:])
            wc2_sb = cpool.tile([32, 2], BF)
            nc.sync.dma_start(wc2_sb[:], wc2[:])
            w1cat_sb = cpool.tile([128, 2, 288], BF)
            nc.sync.dma_start(w1cat_sb[:], w1cat.rearrange("c p f -> p c f"))
            w1dst_sb = cpool.tile([128, 2, 16], BF)
            nc.sync.dma_start(w1dst_sb[:], w1dst.rearrange("c p f -> p c f"))
            w2cat_sb = cpool.tile([128, 1040], BF)
            nc.sync.dma_start(w2cat_sb[:], w2cat[:])
            w2dst_sb = cpool.tile([128, 16], BF)
            nc.sync.dma_start(w2dst_sb[:], w2dst[:])
            scat_sb = {}
            sdti_sb = {}
            for t in "ab":
                scat_sb[t] = cpool.tile([128, NSC], I16, tag=f"scat{t}", name=f"scatsb{t}")
                nc.sync.dma_start(scat_sb[t][:], scat_d[t][:])
                sdti_sb[t] = cpool.tile([128, 2 * NSC], I16, tag=f"sdti{t}", name=f"sdtisb{t}")
                nc.sync.dma_start(sdti_sb[t][:], sdti_d[t][:])
            gidx_sb = {}
            for t in "ab":
                ncalls = scheds[t]["TCpad"] // NCH
                gidx_sb[t] = cpool.tile([128, ncalls, NCH * 8], I16,
                                        tag=f"gidx{t}", name=f"gidxsb{t}")
                nc.scalar.dma_start(gidx_sb[t][:],
                                    gidx_d[t].rearrange("c p s -> p c s"))

            for ti in range(2):
                nc.sync.dma_start(table1cat[ti, N:N + 1, :], poison1[:])
                nc.sync.dma_start(table2cat[ti, N:N + 1, :], poison2[:])

            NT = (ND + 127) // 128    # 20 tiles of local dst rows

            # ---- zero accumulators + sdtab poison rows ----
            with tc.tile_pool(name="zacc", bufs=1) as zaccp:
                zt = zaccp.tile([128, NCC, 128], F32)
                nc.vector.memset(zt[:], 0.0)
                nc.sync.dma_start(acc1.rearrange("(a p) c -> p a c", p=128), zt[:])
                nc.sync.dma_start(acc2.rearrange("(a p) c -> p a c", p=128),
                                  zt[:, :, 0:64])
                zb = zaccp.tile([1, 128], BF)
                nc.vector.memset(zb[:], 0.0)
                nc.sync.dma_start(sdtab2[ND:ND + 1, :], zb[:])

            # ---- sdt1s: slot-ordered dst scores for L1, computed straight
            # into SBUF (no DRAM round trip, no gathers) ----
            sdt1s = cpool.tile([128, NG, 16], BF, tag="sdt1s")
            with tc.tile_pool(name="sd1", bufs=3) as sd1p, \
                 tc.tile_pool(name="sd1ps", bufs=2, space="PSUM") as sd1ps:
                xds_sb = sd1p.tile([128, 2, 2, 2 * NSLOT], BF, tag="xd", bufs=1)
                nc.scalar.dma_start(xds_sb[:],
                                    xds.rearrange("t c p s -> p t c s"))
                for i in range(2 * NSLOT // 128):
                    lo = i * 128
                    for ti in range(2):
                        ps = sd1ps.tile([128, 8], F32, tag="sd1ps")
                        nc.tensor.matmul(
                            out=ps[:], lhsT=xds_sb[:, ti, 0, lo:lo + 128],
                            rhs=w1dst_sb[:, 0, ti * 8:ti * 8 + 8],
                            start=True, stop=False)
                        nc.tensor.matmul(
                            out=ps[:], lhsT=xds_sb[:, ti, 1, lo:lo + 128],
                            rhs=w1dst_sb[:, 1, ti * 8:ti * 8 + 8],
                            start=False, stop=True)
                        nc.vector.tensor_copy(
                            out=sdt1s[:, i, ti * 8:ti * 8 + 8], in_=ps[:])

            # ---- phase 1: layer-1 tables (types fused) ----
            with tc.tile_pool(name="ph1", bufs=4) as p1p, \
                 tc.tile_pool(name="ph1ps", bufs=3, space="PSUM") as p1ps:
                xt_sb = [p1p.tile([128, N], BF, tag=f"xt{c}", name=f"xtsb{c}", bufs=1) for c in range(2)]
                nc.scalar.dma_start(xt_sb[0][:], xT[0])
                nc.sync.dma_start(xt_sb[1][:], xT[1])
                for i in range((N + 127) // 128):
                    lo = i * 128
                    m = min(128, N - lo)
                    ps = p1ps.tile([128, 288], F32, tag="t1ps")
                    nc.tensor.matmul(out=ps[:m], lhsT=xt_sb[0][:, lo:lo + m],
                                     rhs=w1cat_sb[:, 0, :], start=True, stop=False)
                    nc.tensor.matmul(out=ps[:m], lhsT=xt_sb[1][:, lo:lo + m],
                                     rhs=w1cat_sb[:, 1, :], start=False, stop=True)
                    o = p1p.tile([128, 288], BF, tag="t1o")
                    if i % 2 == 0:
                        nc.scalar.copy(out=o[:m], in_=ps[:m])
                    else:
                        nc.vector.tensor_copy(out=o[:m], in_=ps[:m])
                    eng = nc.sync if i % 2 == 0 else nc.scalar
                    eng.dma_start(
                        table1cat[:, lo:lo + m, 0:144].rearrange(
                            "t p f -> p t f"),
                        o[:m].rearrange("p (t f) -> p t f", t=2))

            # ---- edge phase ----
            def gather_sdt(layer, pool, sdtab):
                sdt = {}
                for t in "ab":
                    sdt[t] = pool.tile([128, NG, 128], BF, tag=f"sdt{t}",
                                       name=f"sdt{layer}{t}", bufs=1)
                    sdone = 0
                    while sdone < 2 * NSLOT:
                        n = min(1024, 2 * NSLOT - sdone)
                        nc.gpsimd.dma_gather(
                            sdt[t][:, sdone // 128:(sdone + n) // 128, :],
                            sdtab[:],
                            sdti_sb[t][:, sdone // 16:(sdone + n) // 16],
                            n, n, 128, elem_step=128, queue_num=qrr(t))
                        sdone += n
                return sdt

            def edge_phase(layer, tables, parks, sdt, acc, gbufs):
                CT = T1C if layer == 1 else T2C
                C = 128 if layer == 1 else 512
                SC = C
                hb = C // 8
                NV = 128 // GD

                with tc.tile_pool(name=f"eg{layer}", bufs=gbufs) as gp, \
                     tc.tile_pool(name=f"ew{layer}", bufs=4) as wp, \
                     tc.tile_pool(name=f"es{layer}", bufs=6) as sp, \
                     tc.tile_pool(name=f"ef{layer}", bufs=2) as fp, \
                     tc.tile_pool(name=f"eps{layer}", bufs=2, space="PSUM") as pp, \
                     tc.tile_pool(name=f"ezs{layer}", bufs=2, space="PSUM") as zp:

                    st = {t: dict(call=-1, G=None, Wb=None, pa=None, pz=None)
                          for t in "ab"}

                    def do_group(t, g):
                        ti = 0 if t == "a" else 1
                        sched = scheds[t]
                        cg = int(sched["Rg"][g] // (128 // GD))
                        base = int(sched["cbase"][g])
                        s_ = st[t]
                        if g % NV == 0:
                            s_["pa"] = pp.tile([128, C], F32, tag=f"pa{t}",
                                               name=f"pa{layer}{t}")
                            s_["pz"] = zp.tile([128, NCH, 8], F32, tag=f"pz{t}",
                                               name=f"pz{layer}{t}")
                            nc.vector.memset(s_["pz"][:], 0.0)
                        pa, pz = s_["pa"], s_["pz"]
                        row0 = GD * (g % NV)
                        done = 0
                        while done < cg:
                            seg = min(NCH - (base + done) % NCH, cg - done)
                            call = (base + done) // NCH
                            coff = (base + done) % NCH
                            if call != s_["call"]:
                                G = gp.tile([128, NCH, CT], BF, tag=f"G{t}",
                                            name=f"G{layer}{t}")
                                nc.gpsimd.dma_gather(
                                    G[:, :, :], tables[t][:],
                                    gidx_sb[t][:, call, :],
                                    NCH * 128, NCH * 128, CT,
                                    queue_num=qrr(t))
                                s_["call"] = call
                                s_["G"] = G
                            G = s_["G"]
                            M = G
                            sl = slice(coff, coff + seg)
                            sview = G[:, sl, SC:SC + 8]
                            u = sp.tile([128, NCH, 8], F32, tag=f"u{t}",
                                        name=f"u{layer}{t}")
                            nc.vector.tensor_tensor(
                                out=u[:, :seg, :], in0=sview,
                                in1=sdt[t][:, g, ti * 8:ti * 8 + 8][:, None, :]
                                    .to_broadcast([128, seg, 8]),
                                op=OP.add)
                            phi = sp.tile([128, NCH, 8], F32, tag=f"phi{t}",
                                          name=f"phi{layer}{t}")
                            nc.vector.scalar_tensor_tensor(
                                out=phi[:, :seg, :], in0=u[:, :seg, :], scalar=NEG,
                                in1=u[:, :seg, :], op0=OP.mult, op1=OP.max)
                            q = sp.tile([128, NCH, 8], BF, tag=f"q{t}",
                                        name=f"q{layer}{t}")
                            nc.scalar.activation(out=q[:, :seg, :],
                                                 in_=phi[:, :seg, :], func=AF.Exp)
                            nc.vector.tensor_tensor(
                                out=M[:, sl, 0:C].rearrange(
                                    "p s (c h) -> p s c h", h=8),
                                in0=M[:, sl, 0:C].rearrange(
                                    "p s (c h) -> p s c h", h=8),
                                in1=q[:, :seg, None, :].to_broadcast(
                                    [128, seg, hb, 8]),
                                op=OP.mult)
                            nc.tensor.matmul(
                                out=pz[row0:row0 + GD, :seg, :],
                                lhsT=sconst_sb[:], rhs=q[:, :seg, :],
                                start=False, stop=(done + seg == cg),
                                skip_group_check=True)
                            for s in range(seg):
                                cc = done + s
                                nc.tensor.matmul(
                                    out=pa[row0:row0 + GD, :],
                                    lhsT=sconst_sb[:], rhs=M[:, coff + s, 0:C],
                                    start=(cc == 0), stop=(cc == cg - 1),
                                    skip_group_check=True)
                            done += seg
                        if g % NV == NV - 1:
                            mi = (g * GD) // 128
                            zs = sp.tile([128, 8], F32, tag=f"zs{t}",
                                         name=f"zs{layer}{t}")
                            nc.vector.tensor_reduce(
                                out=zs[:, :, None],
                                in_=pz.rearrange("p s h -> p h s"),
                                axis=AX.X, op=OP.add)
                            z8 = sp.tile([128, 8], F32, tag=f"z8{t}",
                                         name=f"z8{layer}{t}")
                            nc.vector.tensor_scalar(
                                out=z8[:], in0=zs[:],
                                scalar1=(1.0 if layer == 1 else 8.0),
                                scalar2=1e-30, op0=OP.mult, op1=OP.max)
                            rz = sp.tile([128, 8], F32, tag=f"rz{t}",
                                         name=f"rz{layer}{t}")
                            nc.vector.reciprocal(out=rz[:], in_=z8[:])
                            if layer == 1:
                                nc.vector.tensor_tensor(
                                    out=parks[t][:, mi, :].rearrange(
                                        "p (c h) -> p c h", h=8),
                                    in0=pa[:, 0:128].rearrange(
                                        "p (c h) -> p c h", h=8),
                                    in1=rz[:, None, :].to_broadcast(
                                        [128, 16, 8]),
                                    op=OP.mult)
                            else:
                                tmp = fp.tile([128, 512], F32, tag=f"tmp{t}",
                                              name=f"tmp{layer}{t}")
                                nc.vector.tensor_tensor(
                                    out=tmp[:].rearrange("p (c h) -> p c h", h=8),
                                    in0=pa[:].rearrange("p (c h) -> p c h", h=8),
                                    in1=rz[:, None, :].to_broadcast(
                                        [128, 64, 8]),
                                    op=OP.mult)
                                nc.vector.tensor_reduce(
                                    out=parks[t][:, mi, :, None],
                                    in_=tmp[:].rearrange("p (c h) -> p c h", h=8),
                                    axis=AX.X, op=OP.add)

                    CA = 128 if layer == 1 else 64
                    windows = {15: (0, 8), 31: (8, 8), 39: (16, 4)}
                    for g in range(NG):
                        for t in "ab":
                            do_group(t, g)
                        if g in windows:
                            c0, nb = windows[g]
                            for t in "ab":
                                nc.gpsimd.dma_scatter_add(
                                    acc[:], parks[t][:, c0:c0 + nb, :],
                                    scat_sb[t][:, c0 * 8:(c0 + nb) * 8],
                                    nb * 128, nb * 128, CA, queue_num=qrr(t))

            # ---- layer-1 edges ----
            with tc.tile_pool(name="park1", bufs=1) as parkp:
                parks = {t: parkp.tile([128, NCC, 128], F32, tag=f"park{t}",
                                       name=f"park1{t}") for t in "ab"}
                edge_phase(1, {"a": table1cat[0], "b": table1cat[1]}, parks,
                           {"a": sdt1s, "b": sdt1s}, acc1, gbufs=8)

            # ---- combine + ELU helper ----
            def elu_combine(src_ap, cols, tilepool, dst_write):
                for i in range(NT):
                    lo = i * 128
                    m = min(128, ND - lo)
                    a = tilepool.tile([128, cols], F32, tag="ec_a")
                    nc.sync.dma_start(a[:m], src_ap[lo:lo + m, :])
                    e = tilepool.tile([128, cols], F32, tag="ec_e")
                    nc.scalar.activation(out=e[:m], in_=a[:m], func=AF.Exp, scale=0.5)
                    em1 = tilepool.tile([128, cols], F32, tag="ec_em1")
                    nc.vector.tensor_scalar(out=em1[:m], in0=e[:m], scalar1=-1.0,
                                            scalar2=None, op0=OP.add)
                    xm = tilepool.tile([128, cols], F32, tag="ec_xm")
                    nc.vector.tensor_scalar(out=xm[:m], in0=a[:m], scalar1=0.5,
                                            scalar2=None, op0=OP.mult)
                    mk = tilepool.tile([128, cols], mybir.dt.uint8, tag="ec_mk")
                    nc.vector.tensor_scalar(out=mk[:m], in0=a[:m], scalar1=0.0,
                                            scalar2=None, op0=OP.is_gt)
                    h = tilepool.tile([128, cols], BF, tag="ec_h")
                    nc.vector.select(out=h[:m], mask=mk[:m], on_true=xm[:m],
                                     on_false=em1[:m])
                    dst_write(i, lo, m, h)

            # L1 combine -> transposed slice; also L2 dst scores -> sdtab2;
            # AllGather fires per finished column chunk.
            with tc.tile_pool(name="elu1", bufs=4) as elup, \
                 tc.tile_pool(name="elu1ps", bufs=3, space="PSUM") as elups:
                def wr1(i, lo, m, h):
                    c = i // 10
                    off = (i % 10) * 128
                    tps = elups.tile([128, 128], BF, tag="e_tp")
                    nc.tensor.transpose(out=tps[:, :m], in_=h[:m, :],
                                        identity=id_sb[:m, :m])
                    ht = elup.tile([128, 128], BF, tag="e_ht")
                    nc.scalar.copy(out=ht[:, :m], in_=tps[:, :m])
                    nc.scalar.dma_start(h2sliceT[c, :, off:off + m], ht[:, :m])
                    ps2 = elups.tile([128, 16], F32, tag="e_sd2")
                    nc.tensor.matmul(out=ps2[:m], lhsT=ht[:, :m],
                                     rhs=w2dst_sb[:], start=True, stop=True)
                    o2 = elup.tile([128, 16], BF, tag="e_sd2o")
                    nc.vector.tensor_copy(out=o2[:m], in_=ps2[:m])
                    nc.scalar.dma_start(sdtab2[lo:lo + m, 0:16], o2[:m])
                    if i % 10 == 9 or i == NT - 1:
                        nc.gpsimd.collective_compute(
                            "AllGather", mybir.AluOpType.bypass,
                            replica_groups=[list(range(NCORES))],
                            ins=[h2sliceT[c].opt()], outs=[h2fullT[c].opt()])
                elu_combine(acc1[:, :], 128, elup, wr1)

            sdt2ctx = tc.tile_pool(name="sdt2p", bufs=1)
            sdt2p = sdt2ctx.__enter__()
            sdt2 = gather_sdt(2, sdt2p, sdtab2)

            # ---- phase 4: layer-2 tables from SBUF-resident h2T (fused) ----
            with tc.tile_pool(name="ph4", bufs=6) as p4p, \
                 tc.tile_pool(name="ph4ps", bufs=3, space="PSUM") as p4ps, \
                 tc.tile_pool(name="ph4px", bufs=2, space="PSUM") as p4px:
                for c in range(NAG):
                    h2t_sb = p4p.tile([128, NCORES, AGC], BF, tag="h2t",
                                      name="h2tsb", bufs=2)
                    nc.sync.dma_start(h2t_sb[:],
                                      h2fullT[c].rearrange("k p j -> p k j"))
                    ctiles = min(10, (ND - c * AGC + 127) // 128)
                    for k8 in range(NCORES):
                        for jj in range(ctiles):
                            off = jj * 128
                            m = min(128, ND - c * AGC - off)
                            row = k8 * ND + c * AGC + off
                            lhs = h2t_sb[:, k8, off:off + m]
                            psa = p4ps.tile([128, 512], F32, tag="t2psa")
                            nc.tensor.matmul(out=psa[:m], lhsT=lhs,
                                             rhs=w2cat_sb[:, 0:512],
                                             start=True, stop=True)
                            psb = p4ps.tile([128, 512], F32, tag="t2psb")
                            nc.tensor.matmul(out=psb[:m], lhsT=lhs,
                                             rhs=w2cat_sb[:, 512:1024],
                                             start=True, stop=True)
                            psx = p4px.tile([128, 16], F32, tag="t2psx")
                            nc.tensor.matmul(out=psx[:m], lhsT=lhs,
                                             rhs=w2cat_sb[:, 1024:1040],
                                             start=True, stop=True)
                            o2 = p4p.tile([128, 2, 520], BF, tag="t2o")
                            nc.scalar.copy(out=o2[:m, 0, 0:512], in_=psa[:m])
                            nc.vector.tensor_copy(out=o2[:m, 1, 0:512],
                                                  in_=psb[:m])
                            nc.scalar.copy(out=o2[:m, 0, 512:520],
                                           in_=psx[:m, 0:8])
                            nc.vector.tensor_copy(out=o2[:m, 1, 512:520],
                                                  in_=psx[:m, 8:16])
                            eng = nc.sync if jj % 2 == 0 else nc.scalar
                            eng.dma_start(
                                table2cat[:, row:row + m, 0:520].rearrange(
                                    "t p f -> p t f"),
                                o2[:m])

            # ---- layer-2 edges ----
            with tc.tile_pool(name="park2", bufs=1) as park2p:
                parks = {t: park2p.tile([128, NCC, 64], F32, tag=f"park2{t}",
                                        name=f"park2{t}") for t in "ab"}
                edge_phase(2, {"a": table2cat[0], "b": table2cat[1]}, parks,
                           sdt2, acc2, gbufs=6)
            sdt2ctx.__exit__(None, None, None)

            # ---- classifier ----
            with tc.tile_pool(name="cls", bufs=4) as clsp, \
                 tc.tile_pool(name="clsps", bufs=2, space="PSUM") as clsps:
                def wrc(i, lo, m, h):
                    tps = clsps.tile([64, 128], BF, tag="c_t1")
                    nc.tensor.transpose(out=tps[:, :m], in_=h[:m, :],
                                        identity=id_sb[:m, :m])
                    h3t = clsp.tile([64, 128], BF, tag="c_h3t")
                    nc.scalar.copy(out=h3t[:, :m], in_=tps[:, :m])
                    z1 = clsps.tile([128, 32], F32, tag="c_z1")
                    nc.tensor.matmul(out=z1[:m], lhsT=h3t[:, :m], rhs=wc1_sb[:],
                                     start=True, stop=True)
                    z1s = clsp.tile([128, 32], BF, tag="c_z1s")
                    nc.scalar.activation(out=z1s[:m], in_=z1[:m], func=AF.Relu)
                    t2ps = clsps.tile([32, 128], BF, tag="c_t2")
                    nc.tensor.transpose(out=t2ps[:, :m], in_=z1s[:m, :],
                                        identity=id_sb[:m, :m])
                    z1t = clsp.tile([32, 128], BF, tag="c_z1t")
                    nc.scalar.copy(out=z1t[:, :m], in_=t2ps[:, :m])
                    lg = clsps.tile([128, 2], F32, tag="c_lg")
                    nc.tensor.matmul(out=lg[:m], lhsT=z1t[:, :m], rhs=wc2_sb[:],
                                     start=True, stop=True)
                    lo_ = clsp.tile([128, 2], F32, tag="c_out")
                    nc.vector.tensor_copy(out=lo_[:m], in_=lg[:m])
                    nc.sync.dma_start(out[lo:lo + m, :], lo_[:m])
                elu_combine(acc2[:, :], 64, clsp, wrc)

    nc.compile()
    return nc


# ----------------------------------------------------------------------------
# entry point
# ----------------------------------------------------------------------------

_CACHE = {}


def _prepare(inputs):
    per_core, sched_a, sched_b = _host_prep(inputs)
    key = (sched_a["TCpad"], sched_b["TCpad"],
           tuple(sched_a["Rg"]), tuple(sched_b["Rg"]))
    if key not in _CACHE:
        _CACHE.clear()
        _CACHE[key] = _build_nc(sched_a, sched_b)
    return _CACHE[key], per_core


def _run(nc, per_core, **kw):
    from concourse import bass_utils
    return bass_utils.run_bass_kernel_spmd(nc, per_core,
                                           core_ids=list(range(NCORES)), **kw)


def kernel(**inputs):
    nc, per_core = _prepare(inputs)
    res = _run(nc, per_core)
    return np.concatenate([res.results[k]["out"] for k in range(NCORES)], 0)
